# revision 18
# baseline (speedup 1.0000x reference)
"""Distributed attention kernel for 8 TRN2 NeuronCores.

Problem: L=2048, B=2, E=256, H=8 heads, D=32 head-dim, fp32.

Sharding: DP2 over batch x sequence-parallel-4 over query positions.
Core c handles batch c//4, query rows [512*(c%4), 512*(c%4+1)), ALL 8
heads. k/v projections are redundantly computed per batch group (cheap)
and NO collective is needed: each core owns a disjoint output block.

Per-core pipeline (v4 -- cell units, ping-pong score tiles, dual exp):
  - The score work is 128 cells (pass, tk-chunk g, head h) of
    [K=32 d, M=128 tk, N=512 tq]. Cells run THREE at a time as one
    "unit": 3 concurrent PE matmuls on distinct 32-row bands
    (tile_position row tiling), each filling its own PSUM bank (a bank
    shared by concurrently-executing row-tiled matmuls hangs the
    device -- HW-verified).
  - TWO 3-bank score tiles ping-pong between units. Separate pool
    tiles are required: the Tile dep tracker is coarse-grained, so a
    shared tile serializes unit i+1's scores behind unit i's exp read
    (measured +0.7us/unit).
  - softmax exp runs on TWO engines in parallel: ScalarE exact exp via
    the ACT LUT; VectorE a Schraudolph exp2 (one fused mult+add
    tensor_scalar emitting the bf16 BIT PATTERN as int16, ~1.8% rms
    error, softmax-normalized). A minority of units take the DVE path
    so the output error stays ~1.3% (budget 2e-2).
  - PV is software-pipelined one unit behind (the PE is in-order; a PV
    waiting on exp would head-of-line-block the next scores), and
    deferred entirely while the q/k/v projections own the last two
    PSUM banks -- pool lifetimes let the projection psum, the PV
    accumulators, and the final projection accumulators share banks
    6-7 in sequence.
  - PV uses P.T chunks as STATIONARY and [v|1] as moving so O lands in
    natural [tq, d] orientation with the softmax denominator Z as a
    free per-partition column; xbar DMA transposes produce O.T, and
    the Wp projection runs at the tail with the per-head 1/Z folded in
    beforehand (reciprocal + broadcast multiply on DVE).
"""

import math
import os
import sys

import numpy as np

for _p in ("/opt/trn_rl_repo",):
    if _p not in sys.path and os.path.isdir(_p):
        sys.path.insert(0, _p)

import ml_dtypes

import concourse.bass as bass
import concourse.bacc as bacc
import concourse.mybir as mybir
import concourse.tile as tile
from concourse.bass_utils import run_bass_kernel_spmd

dt = mybir.dt
F32 = dt.float32
BF16 = dt.bfloat16
I16 = dt.int16
AF = mybir.ActivationFunctionType
ALU = mybir.AluOpType
BF = ml_dtypes.bfloat16

L, B, E, H, D = 2048, 2, 256, 8, 32
SCALE = float(D) ** -0.5
NCORES = 8
SP = 4            # sequence-parallel ways
TQ = L // SP      # 512 query rows per core
NTK = L // 128    # 16 tk chunks
VW = H * (D + 1)  # v_buf cols per tk chunk: 8x [v_h | 1] = 264
NPASS = 2         # head passes (4 heads each)
CPU = 3           # cells per unit

# Schraudolph exp2: bf16 bits of exp(s*SCALE) ~= int16(s*C1 + C2).
C1 = 128.0 * SCALE * math.log2(math.e)
C2 = 128.0 * (127.0 - 0.0434) + 0.5

_GRAPH = None


def _build_graph():
    nc = bacc.Bacc(
        "TRN2",
        target_bir_lowering=False,
        debug=False,
        enable_asserts=False,
        num_devices=NCORES,
    )

    xqt = nc.declare_dram_parameter("xqt", [E, TQ], BF16, isOutput=False).ap()
    xkt = nc.declare_dram_parameter("xkt", [E, L], BF16, isOutput=False).ap()
    xvt = nc.declare_dram_parameter("xvt", [E, L], BF16, isOutput=False).ap()
    wq = nc.declare_dram_parameter("wq", [E, E], BF16, isOutput=False).ap()
    wk = nc.declare_dram_parameter("wk", [E, E], BF16, isOutput=False).ap()
    wv = nc.declare_dram_parameter("wv", [E, E], BF16, isOutput=False).ap()
    wp = nc.declare_dram_parameter("wp", [E, E], BF16, isOutput=False).ap()
    bq = nc.declare_dram_parameter("bq", [1, E], F32, isOutput=False).ap()
    bk = nc.declare_dram_parameter("bk", [1, E], F32, isOutput=False).ap()
    bv = nc.declare_dram_parameter("bv", [1, E], F32, isOutput=False).ap()
    bp = nc.declare_dram_parameter("bp", [1, E], F32, isOutput=False).ap()
    out = nc.declare_dram_parameter("out", [TQ, E], F32, isOutput=True).ap()

    with tile.TileContext(nc) as tc:
        with (
            tc.tile_pool(name="persist", bufs=1) as pp,
            tc.tile_pool(name="pt", bufs=26) as ptp,
            tc.tile_pool(name="osb", bufs=2) as osbp,
            tc.tile_pool(name="onat", bufs=2) as onatp,
            tc.tile_pool(name="rz", bufs=2) as rzp,
            tc.tile_pool(name="vstage", bufs=4) as vsp,
            tc.tile_pool(name="outsb", bufs=2) as outp,
            tc.tile_pool(name="st", bufs=2, space="PSUM") as stp,
        ):
            # ---------- phase 0: loads ----------
            # weights: tile [128, 2E]; slice e covers W rows [128e, 128e+128)
            w_sb = {}

            def load_w(name, wsrc, eng):
                t = pp.tile([128, 2 * E], BF16, name=f"w{name}", tag=f"w{name}")
                eng.dma_start(
                    out=t[:].rearrange("p (e n) -> p e n", e=2),
                    in_=wsrc.rearrange("(e p) n -> p e n", p=128),
                )
                w_sb[name] = t

            # scalar queue: q-projection inputs first; sync queue: wk +
            # half of xk. v inputs and remaining weights come later.
            load_w("q", wq, nc.scalar)
            load_w("k", wk, nc.sync)
            xq_sb = []
            for e in range(2):
                t = pp.tile([128, TQ], BF16, name=f"xqt{e}", tag=f"xqt{e}")
                nc.scalar.dma_start(out=t[:], in_=xqt[e * 128:(e + 1) * 128, :])
                xq_sb.append(t)
            xk_sb = [
                pp.tile([128, L], BF16, name=f"xkt{e}", tag=f"xkt{e}")
                for e in range(2)
            ]
            for n in range(4):
                for e in range(2):
                    eng = nc.scalar if e == 0 else nc.sync
                    eng.dma_start(
                        out=xk_sb[e][:, n * 512:(n + 1) * 512],
                        in_=xkt[e * 128:(e + 1) * 128, n * 512:(n + 1) * 512],
                    )
            bq_sb = pp.tile([128, 2], F32)
            nc.gpsimd.dma_start(
                out=bq_sb[:], in_=bq.rearrange("a (c p) -> p (a c)", p=128)
            )
            bv_col = pp.tile([128, 2], BF16)
            nc.gpsimd.dma_start(
                out=bv_col[:], in_=bv.rearrange("a (c p) -> p (a c)", p=128)
            )
            bp_sb = pp.tile([128, E], F32)
            nc.gpsimd.dma_start(out=bp_sb[:], in_=bp.to_broadcast((128, E)))
            load_w("v", wv, nc.gpsimd)
            load_w("p", wp, nc.gpsimd)
            xv_sb = [
                pp.tile([128, L], BF16, name=f"xvt{e}", tag=f"xvt{e}")
                for e in range(2)
            ]
            for n in range(4):
                for e in range(2):
                    eng = nc.scalar if e == 0 else nc.sync
                    eng.dma_start(
                        out=xv_sb[e][:, n * 512:(n + 1) * 512],
                        in_=xvt[e * 128:(e + 1) * 128, n * 512:(n + 1) * 512],
                    )

            # warm the exp ACT table AFTER the load triggers are on the
            # scalar queue (the ~2.7us table load must not delay them)
            warm = pp.tile([1, 16], F32)
            nc.vector.memset(warm[:], 0.0)
            nc.scalar.activation(warm[:], warm[:], AF.Exp)

            # ---------- persistent SBUF state ----------
            # kT[hc]: [128 = 4 heads x 32 d (bands 0/32/64/96), 2048 tk]
            kT = [pp.tile([128, L], BF16, name=f"kT{hc}", tag=f"kT{hc}")
                  for hc in range(2)]
            qT = [pp.tile([128, TQ], BF16, name=f"qT{hc}", tag=f"qT{hc}")
                  for hc in range(2)]
            v_buf = pp.tile([128, NTK * VW], BF16)
            nc.gpsimd.memset(v_buf[:], 1.0)

            # ping-pong score tiles: 3 banks each (bank r <-> the r-th
            # row band used by the unit)
            st_ab = [
                stp.tile([128, CPU * 512], F32, name=f"st{i}", tag="st")
                for i in range(2)
            ]

            # ---------- cell/unit machinery ----------
            # cell = (pass, g, h): scores for head 4p+h over tk chunk g,
            # all 512 tq. Units take 3 consecutive cells (distinct h mod
            # 4 -> distinct PE row bands).
            cells = [(p, g, h) for p in range(NPASS) for g in range(NTK)
                     for h in range(4)]
            cursor = [0]        # next cell index
            unit_no = [0]
            pv_pending = []     # descs awaiting PV emission
            pv_enabled = [False]
            po_tiles = {}

            def emit_pv_cell(desc):
                p, g, h, pt, r = desc
                poA, poB = po_tiles[p]
                po = poA if h < 2 else poB
                uu = h % 2
                hh = 4 * p + h
                for m in range(4):
                    nc.tensor.matmul(
                        po[:, uu * 132 + m * 33: uu * 132 + m * 33 + 33],
                        pt[:, r * 512 + m * 128: r * 512 + (m + 1) * 128],
                        v_buf[:, g * VW + hh * (D + 1): g * VW + (hh + 1) * (D + 1)],
                        start=(g == 0 and uu == 0 and m == 0),
                        stop=(g == NTK - 1 and uu == 1 and m == 3),
                        skip_group_check=True,
                    )

            def flush_pv(keep=0):
                while len(pv_pending) > keep:
                    emit_pv_cell(pv_pending.pop(0))

            def emit_unit():
                """scores + exp for the next <=3 cells; queues their PV."""
                lo = cursor[0]
                hi = min(lo + CPU, len(cells))
                if lo >= hi:
                    return False
                cursor[0] = hi
                q = unit_no[0]
                unit_no[0] += 1
                st = st_ab[q % 2]
                ncell = hi - lo
                for r in range(ncell):
                    p, g, h = cells[lo + r]
                    nc.tensor.matmul(
                        st[:, r * 512:(r + 1) * 512],
                        kT[p][32 * h:32 * h + D, g * 128:(g + 1) * 128],
                        qT[p][32 * h:32 * h + D, :],
                        start=True,
                        stop=True,
                        tile_position=(32 * h, 0),
                    )
                pt = ptp.tile([128, CPU * 512], BF16, tag="pt")
                # exp split WITHIN the unit: ScalarE takes the first two
                # cells (exact exp), DVE the third (Schraudolph). Both
                # run concurrently, so the unit's exp latency is the
                # ScalarE instruction (~1.1us), which fits under the
                # two-unit PE budget of the st-tile ping-pong chain.
                ws = min(2, ncell) * 512
                nc.scalar.activation(
                    pt[:, 0:ws], st[:, 0:ws], AF.Exp, scale=SCALE
                )
                if ncell == CPU:
                    nc.vector.tensor_scalar(
                        pt[:, ws:ws + 512].bitcast(I16), st[:, ws:ws + 512],
                        C1, C2, ALU.mult, ALU.add,
                    )
                for r in range(ncell):
                    p, g, h = cells[lo + r]
                    pv_pending.append((p, g, h, pt, r))
                if pv_enabled[0]:
                    flush_pv(keep=CPU)
                return True

            # ---------- projections (psum banks 6-7), interleaved with
            # the first attention units' scores+exp (PV deferred) ----------
            with tc.tile_pool(name="ps", bufs=2, space="PSUM") as psq:
                for hc in range(2):
                    ps = psq.tile([128, TQ], F32, tag="ps")
                    for e in range(2):
                        nc.tensor.matmul(
                            ps[:],
                            w_sb["q"][:, e * E + hc * 128: e * E + (hc + 1) * 128],
                            xq_sb[e][:, :],
                            start=(e == 0),
                            stop=(e == 1),
                        )
                    nc.vector.tensor_scalar_add(
                        qT[hc][:, :], ps[:], bq_sb[:, hc:hc + 1]
                    )
                for n in range(4):
                    for hc in range(2):
                        ps = psq.tile([128, 512], F32, tag="ps")
                        for e in range(2):
                            nc.tensor.matmul(
                                ps[:],
                                w_sb["k"][:, e * E + hc * 128: e * E + (hc + 1) * 128],
                                xk_sb[e][:, n * 512:(n + 1) * 512],
                                start=(e == 0),
                                stop=(e == 1),
                            )
                        # bk dropped: softmax(S + const-per-row) is
                        # invariant, and (q+bq).bk is constant across
                        # keys. Pure copy -> ScalarE (DVE is the busier
                        # engine early on).
                        nc.scalar.activation(
                            kT[hc][:, n * 512:(n + 1) * 512], ps[:], AF.Copy
                        )
                    for t in range(4 * n, 4 * n + 4):
                        ps = psq.tile([128, E], F32, tag="ps")
                        for e in range(2):
                            nc.tensor.matmul(
                                ps[:],
                                xv_sb[e][:, t * 128:(t + 1) * 128],
                                w_sb["v"][:, e * E:(e + 1) * E],
                                start=(e == 0),
                                stop=(e == 1),
                            )
                        vs = vsp.tile([128, E], BF16, tag="vstage")
                        # bv folds into the output bias (sum of softmax
                        # weights is 1): out += bv @ Wp, added at the
                        # tail. Pure copy -> ScalarE.
                        nc.scalar.activation(vs[:], ps[:], AF.Copy)
                        nc.sync.dma_start(
                            out=v_buf[:, t * VW:(t + 1) * VW].rearrange(
                                "p (h w) -> p h w", h=H
                            )[:, :, 0:D],
                            in_=vs[:].rearrange("p (h d) -> p h d", h=H),
                        )
                    # attention units whose kT chunks are now projected:
                    # pass-0 cells with g <= 4n+3
                    while cursor[0] <= (4 * n + 4) * 4 - CPU:
                        emit_unit()

            # ---------- PV accumulators take over banks 6-7 ----------
            onat_t = {}
            osb_t = {}

            def finalize(p):
                """normalize + transpose O for pass p (proj at tail)."""
                poA, poB = po_tiles[p]
                onat = onatp.tile([128, TQ], BF16, name=f"onat{p}", tag="onat")
                osb = osbp.tile([128, TQ], BF16, name=f"osb{p}", tag="osb")
                rz = rzp.tile([128, 16], F32, name=f"rz{p}", tag="rz")
                onat_t[p], osb_t[p] = onat, osb
                for idx, po in ((0, poA), (1, poB)):
                    zv = po[:].rearrange("p (b m w) -> p b m w", b=2, m=4)[
                        :, :, :, D:D + 1
                    ]
                    rzo = rz[:, idx * 8:(idx + 1) * 8].rearrange(
                        "p (b m) -> p b m", b=2
                    ).unsqueeze(3)
                    nc.vector.reciprocal(rzo, zv)
                for u in range(4):
                    po = poA if u < 2 else poB
                    uu = u % 2
                    idx = u // 2
                    pin = po[:].rearrange("p (mm w) -> p mm w", w=33)[
                        :, uu * 4: uu * 4 + 4, 0:D
                    ]
                    rzb = rz[
                        :, idx * 8 + uu * 4: idx * 8 + uu * 4 + 4
                    ].unsqueeze(2).to_broadcast((128, 4, D))
                    pout = onat[:].rearrange(
                        "p (m b w) -> p m b w", m=4, b=4
                    )[:, :, u:u + 1, :]
                    nc.vector.tensor_tensor(pout, pin, rzb, ALU.mult)
                for m in range(4):
                    eng = nc.sync if m % 2 == 0 else nc.scalar
                    eng.dma_start_transpose(
                        osb[:, m * 128:(m + 1) * 128],
                        onat[:, m * 128:(m + 1) * 128],
                    )

            with tc.tile_pool(name="po", bufs=2, space="PSUM") as pop:
                po_tiles[0] = (
                    pop.tile([128, 264], F32, name="poA0", tag="po"),
                    pop.tile([128, 264], F32, name="poB0", tag="po"),
                )
                pv_enabled[0] = True
                flush_pv(keep=CPU)
                # emit remaining pass-0 cells (units may straddle into
                # pass 1; their pass-1 PVs wait in pv_pending)
                npass0_cells = NTK * 4
                while cursor[0] < npass0_cells:
                    emit_unit()
                while any(d[0] == 0 for d in pv_pending):
                    emit_pv_cell(pv_pending.pop(0))
                finalize(0)
                po_tiles[1] = (
                    pop.tile([128, 264], F32, name="poA1", tag="po"),
                    pop.tile([128, 264], F32, name="poB1", tag="po"),
                )
                flush_pv(keep=CPU)
                while emit_unit():
                    pass
                flush_pv()
                finalize(1)

                # ---------- tail: Wp projection + bias + out DMA ----------
                pjt = [
                    pop.tile([128, 2 * E], F32, name=f"pjt{i}", tag="po")
                    for i in range(2)
                ]
                # from the st pool: its slots are dead at the tail (the
                # po pool's 2 slots still hold the live pjt tiles)
                bbp = stp.tile([128, E], F32, name="bbp", tag="st")
                for e in range(2):
                    nc.tensor.matmul(
                        bbp[:],
                        bv_col[:, e:e + 1].to_broadcast((128, 128)),
                        w_sb["p"][:, e * E:(e + 1) * E],
                        start=(e == 0),
                        stop=(e == 1),
                    )
                bb_sb = pp.tile([128, E], F32, name="bb_sb")
                nc.vector.tensor_tensor(bb_sb[:], bbp[:], bp_sb[:], ALU.add)
                for m in range(4):
                    for p in range(NPASS):
                        nc.tensor.matmul(
                            pjt[m // 2][:, (m % 2) * E:(m % 2 + 1) * E],
                            osb_t[p][:, m * 128:(m + 1) * 128],
                            w_sb["p"][:, p * E:(p + 1) * E],
                            start=(p == 0 and m % 2 == 0),
                            stop=(p == NPASS - 1 and m % 2 == 1),
                            skip_group_check=True,
                        )
                for m in range(TQ // 128):
                    ob = outp.tile([128, E], F32, tag="outsb")
                    nc.vector.tensor_tensor(
                        ob[:], pjt[m // 2][:, (m % 2) * E:(m % 2 + 1) * E],
                        bb_sb[:], ALU.add,
                    )
                    eng = nc.sync if m % 2 == 0 else nc.scalar
                    eng.dma_start(
                        out=out[m * 128:(m + 1) * 128, :], in_=ob[:]
                    )

    return nc


def get_graph():
    global _GRAPH
    if _GRAPH is None:
        nc = _build_graph()
        nc.compile()
        _GRAPH = nc
    return _GRAPH


def make_in_maps(query, key_, value, Wq, bq, Wk, bk, Wv, bv, Wp, bp):
    query = np.asarray(query, np.float32)
    key_ = np.asarray(key_, np.float32)
    value = np.asarray(value, np.float32)
    Wq, Wk, Wv, Wp = (np.asarray(w, np.float32) for w in (Wq, Wk, Wv, Wp))
    bq, bk, bv, bp = (np.asarray(b_, np.float32) for b_ in (bq, bk, bv, bp))

    wq_b = np.ascontiguousarray(Wq).astype(BF)
    wk_b = np.ascontiguousarray(Wk).astype(BF)
    wv_b = np.ascontiguousarray(Wv).astype(BF)
    wp_b = np.ascontiguousarray(Wp).astype(BF)
    xt = {}
    for b in range(B):
        xt[("q", b)] = np.ascontiguousarray(query[:, b, :].T).astype(BF)
        xt[("k", b)] = np.ascontiguousarray(key_[:, b, :].T).astype(BF)
        xt[("v", b)] = np.ascontiguousarray(value[:, b, :].T).astype(BF)

    in_maps = []
    for c in range(NCORES):
        b = c // SP
        p = c % SP
        m = {
            "xqt": np.ascontiguousarray(xt[("q", b)][:, p * TQ:(p + 1) * TQ]),
            "xkt": xt[("k", b)],
            "xvt": xt[("v", b)],
            "wq": wq_b,
            "wk": wk_b,
            "wv": wv_b,
            "wp": wp_b,
            "bq": bq.reshape(1, E).copy(),
            "bk": bk.reshape(1, E).copy(),
            "bv": bv.reshape(1, E).copy(),
            "bp": bp.reshape(1, E).copy(),
        }
        in_maps.append(m)
    return in_maps


def assemble(results):
    out_full = np.empty((L, B, E), np.float32)
    for c in range(NCORES):
        b = c // SP
        p = c % SP
        out_full[p * TQ:(p + 1) * TQ, b, :] = results[c]["out"]
    return out_full


def run(inputs, trace=False, **kw):
    nc = get_graph()
    in_maps = make_in_maps(**inputs)
    res = run_bass_kernel_spmd(
        nc, in_maps, core_ids=list(range(NCORES)), trace=trace, **kw
    )
    return res


def kernel(**inputs):
    res = run(inputs, trace=False)
    return assemble(res.results)


# revision 19
# speedup vs baseline: 1.0822x; 1.0822x over previous
"""Distributed attention kernel for 8 TRN2 NeuronCores.

Problem: L=2048, B=2, E=256, H=8 heads, D=32 head-dim, fp32.

Sharding: DP2 over batch x sequence-parallel-4 over query positions.
Core c handles batch c//4, query rows [512*(c%4), 512*(c%4+1)), ALL 8
heads. k/v projections are redundantly computed per batch group (cheap)
and NO collective is needed: each core owns a disjoint output block.

Per-core pipeline (v4 -- cell units, ping-pong score tiles, dual exp):
  - The score work is 128 cells (pass, tk-chunk g, head h) of
    [K=32 d, M=128 tk, N=512 tq]. Cells run THREE at a time as one
    "unit": 3 concurrent PE matmuls on distinct 32-row bands
    (tile_position row tiling), each filling its own PSUM bank (a bank
    shared by concurrently-executing row-tiled matmuls hangs the
    device -- HW-verified).
  - TWO 3-bank score tiles ping-pong between units. Separate pool
    tiles are required: the Tile dep tracker is coarse-grained, so a
    shared tile serializes unit i+1's scores behind unit i's exp read
    (measured +0.7us/unit).
  - softmax exp runs on TWO engines in parallel: ScalarE exact exp via
    the ACT LUT; VectorE a Schraudolph exp2 (one fused mult+add
    tensor_scalar emitting the bf16 BIT PATTERN as int16, ~1.8% rms
    error, softmax-normalized). A minority of units take the DVE path
    so the output error stays ~1.3% (budget 2e-2).
  - PV is software-pipelined one unit behind (the PE is in-order; a PV
    waiting on exp would head-of-line-block the next scores), and
    deferred entirely while the q/k/v projections own the last two
    PSUM banks -- pool lifetimes let the projection psum, the PV
    accumulators, and the final projection accumulators share banks
    6-7 in sequence.
  - PV uses P.T chunks as STATIONARY and [v|1] as moving so O lands in
    natural [tq, d] orientation with the softmax denominator Z as a
    free per-partition column; xbar DMA transposes produce O.T, and
    the Wp projection runs at the tail with the per-head 1/Z folded in
    beforehand (reciprocal + broadcast multiply on DVE).
"""

import math
import os
import sys

import numpy as np

for _p in ("/opt/trn_rl_repo",):
    if _p not in sys.path and os.path.isdir(_p):
        sys.path.insert(0, _p)

import ml_dtypes

import concourse.bass as bass
import concourse.bacc as bacc
import concourse.mybir as mybir
import concourse.tile as tile
from concourse.bass_utils import run_bass_kernel_spmd

dt = mybir.dt
F32 = dt.float32
BF16 = dt.bfloat16
I16 = dt.int16
AF = mybir.ActivationFunctionType
ALU = mybir.AluOpType
BF = ml_dtypes.bfloat16

L, B, E, H, D = 2048, 2, 256, 8, 32
SCALE = float(D) ** -0.5
NCORES = 8
SP = 4            # sequence-parallel ways
TQ = L // SP      # 512 query rows per core
NTK = L // 128    # 16 tk chunks
VW = H * (D + 1)  # v_buf cols per tk chunk: 8x [v_h | 1] = 264
NPASS = 2         # head passes (4 heads each)
CPU = 3           # cells per unit

# Schraudolph exp2: bf16 bits of exp(s*SCALE) ~= int16(s*C1 + C2).
C1 = 128.0 * SCALE * math.log2(math.e)
C2 = 128.0 * (127.0 - 0.0434) + 0.5

_GRAPH = None


def _build_graph():
    nc = bacc.Bacc(
        "TRN2",
        target_bir_lowering=False,
        debug=False,
        enable_asserts=False,
        num_devices=NCORES,
    )

    xqt = nc.declare_dram_parameter("xqt", [E, TQ], BF16, isOutput=False).ap()
    xkt = nc.declare_dram_parameter("xkt", [E, L], BF16, isOutput=False).ap()
    xvt = nc.declare_dram_parameter("xvt", [E, L], BF16, isOutput=False).ap()
    wq = nc.declare_dram_parameter("wq", [E, E], BF16, isOutput=False).ap()
    wk = nc.declare_dram_parameter("wk", [E, E], BF16, isOutput=False).ap()
    wv = nc.declare_dram_parameter("wv", [E, E], BF16, isOutput=False).ap()
    wp = nc.declare_dram_parameter("wp", [E, E], BF16, isOutput=False).ap()
    bq = nc.declare_dram_parameter("bq", [1, E], F32, isOutput=False).ap()
    bk = nc.declare_dram_parameter("bk", [1, E], F32, isOutput=False).ap()
    bv = nc.declare_dram_parameter("bv", [1, E], F32, isOutput=False).ap()
    bp = nc.declare_dram_parameter("bp", [1, E], F32, isOutput=False).ap()
    out = nc.declare_dram_parameter("out", [TQ, E], F32, isOutput=True).ap()

    with tile.TileContext(nc) as tc:
        with (
            tc.tile_pool(name="persist", bufs=1) as pp,
            tc.tile_pool(name="pt", bufs=26) as ptp,
            tc.tile_pool(name="osb", bufs=2) as osbp,
            tc.tile_pool(name="onat", bufs=2) as onatp,
            tc.tile_pool(name="rz", bufs=2) as rzp,
            tc.tile_pool(name="vstage", bufs=4) as vsp,
            tc.tile_pool(name="outsb", bufs=2) as outp,
            tc.tile_pool(name="st", bufs=2, space="PSUM") as stp,
        ):
            # ---------- phase 0: loads ----------
            # weights: tile [128, 2E]; slice e covers W rows [128e, 128e+128)
            w_sb = {}

            def load_w(name, wsrc, eng):
                t = pp.tile([128, 2 * E], BF16, name=f"w{name}", tag=f"w{name}")
                eng.dma_start(
                    out=t[:].rearrange("p (e n) -> p e n", e=2),
                    in_=wsrc.rearrange("(e p) n -> p e n", p=128),
                )
                w_sb[name] = t

            # scalar queue: q-projection inputs first; sync queue: wk +
            # half of xk. v inputs and remaining weights come later.
            load_w("q", wq, nc.scalar)
            load_w("k", wk, nc.sync)
            xq_sb = []
            for e in range(2):
                t = pp.tile([128, TQ], BF16, name=f"xqt{e}", tag=f"xqt{e}")
                nc.scalar.dma_start(out=t[:], in_=xqt[e * 128:(e + 1) * 128, :])
                xq_sb.append(t)
            xk_sb = [
                pp.tile([128, L], BF16, name=f"xkt{e}", tag=f"xkt{e}")
                for e in range(2)
            ]
            for n in range(4):
                for e in range(2):
                    eng = nc.scalar if e == 0 else nc.sync
                    eng.dma_start(
                        out=xk_sb[e][:, n * 512:(n + 1) * 512],
                        in_=xkt[e * 128:(e + 1) * 128, n * 512:(n + 1) * 512],
                    )
            bq_sb = pp.tile([128, 2], F32)
            nc.gpsimd.dma_start(
                out=bq_sb[:], in_=bq.rearrange("a (c p) -> p (a c)", p=128)
            )
            bv_col = pp.tile([128, 2], BF16)
            nc.gpsimd.dma_start(
                out=bv_col[:], in_=bv.rearrange("a (c p) -> p (a c)", p=128)
            )
            bp_sb = pp.tile([128, E], F32)
            nc.gpsimd.dma_start(out=bp_sb[:], in_=bp.to_broadcast((128, E)))
            load_w("v", wv, nc.gpsimd)
            load_w("p", wp, nc.gpsimd)
            xv_sb = [
                pp.tile([128, L], BF16, name=f"xvt{e}", tag=f"xvt{e}")
                for e in range(2)
            ]
            for n in range(4):
                for e in range(2):
                    eng = nc.scalar if e == 0 else nc.sync
                    eng.dma_start(
                        out=xv_sb[e][:, n * 512:(n + 1) * 512],
                        in_=xvt[e * 128:(e + 1) * 128, n * 512:(n + 1) * 512],
                    )

            # warm the exp ACT table AFTER the load triggers are on the
            # scalar queue (the ~2.7us table load must not delay them)
            warm = pp.tile([1, 16], F32)
            nc.vector.memset(warm[:], 0.0)
            nc.scalar.activation(warm[:], warm[:], AF.Exp)

            # ---------- persistent SBUF state ----------
            # kT[hc]: [128 = 4 heads x 32 d (bands 0/32/64/96), 2048 tk]
            kT = [pp.tile([128, L], BF16, name=f"kT{hc}", tag=f"kT{hc}")
                  for hc in range(2)]
            qT = [pp.tile([128, TQ], BF16, name=f"qT{hc}", tag=f"qT{hc}")
                  for hc in range(2)]
            v_buf = pp.tile([128, NTK * VW], BF16)
            nc.gpsimd.memset(v_buf[:], 1.0)

            # ping-pong score tiles: 3 banks each (bank r <-> the r-th
            # row band used by the unit)
            st_ab = [
                stp.tile([128, CPU * 512], F32, name=f"st{i}", tag="st")
                for i in range(2)
            ]

            # ---------- cell/unit machinery ----------
            # cell = (pass, g, h): scores for head 4p+h over tk chunk g,
            # all 512 tq. Units take 3 consecutive cells (distinct h mod
            # 4 -> distinct PE row bands).
            cells = [(p, g, h) for p in range(NPASS) for g in range(NTK)
                     for h in range(4)]
            cursor = [0]        # next cell index
            unit_no = [0]
            pv_pending = []     # descs awaiting PV emission
            pv_enabled = [False]
            po_tiles = {}

            def emit_pv_cell(desc):
                p, g, h, pt, r = desc
                poA, poB = po_tiles[p]
                po = poA if h < 2 else poB
                uu = h % 2
                hh = 4 * p + h
                for m in range(4):
                    nc.tensor.matmul(
                        po[:, uu * 132 + m * 33: uu * 132 + m * 33 + 33],
                        pt[:, r * 512 + m * 128: r * 512 + (m + 1) * 128],
                        v_buf[:, g * VW + hh * (D + 1): g * VW + (hh + 1) * (D + 1)],
                        start=(g == 0 and uu == 0 and m == 0),
                        stop=(g == NTK - 1 and uu == 1 and m == 3),
                        skip_group_check=True,
                    )

            def flush_pv(keep=0):
                while len(pv_pending) > keep:
                    emit_pv_cell(pv_pending.pop(0))

            def emit_unit():
                """scores + exp for the next <=3 cells; queues their PV."""
                lo = cursor[0]
                hi = min(lo + CPU, len(cells))
                if lo >= hi:
                    return False
                cursor[0] = hi
                q = unit_no[0]
                unit_no[0] += 1
                st = st_ab[q % 2]
                ncell = hi - lo
                for r in range(ncell):
                    p, g, h = cells[lo + r]
                    nc.tensor.matmul(
                        st[:, r * 512:(r + 1) * 512],
                        kT[p][32 * h:32 * h + D, g * 128:(g + 1) * 128],
                        qT[p][32 * h:32 * h + D, :],
                        start=True,
                        stop=True,
                        tile_position=(32 * h, 0),
                    )
                pt = ptp.tile([128, CPU * 512], BF16, tag="pt")
                # exp split WITHIN the unit: ScalarE takes the first two
                # cells (exact exp), DVE the third (Schraudolph). Both
                # run concurrently, so the unit's exp latency is the
                # ScalarE instruction (~1.1us), which fits under the
                # two-unit PE budget of the st-tile ping-pong chain.
                ws = min(2, ncell) * 512
                nc.scalar.activation(
                    pt[:, 0:ws], st[:, 0:ws], AF.Exp, scale=SCALE
                )
                if ncell == CPU:
                    nc.vector.tensor_scalar(
                        pt[:, ws:ws + 512].bitcast(I16), st[:, ws:ws + 512],
                        C1, C2, ALU.mult, ALU.add,
                    )
                for r in range(ncell):
                    p, g, h = cells[lo + r]
                    pv_pending.append((p, g, h, pt, r))
                if pv_enabled[0]:
                    flush_pv(keep=CPU)
                return True

            # ---------- projections (psum banks 6-7), interleaved with
            # the first attention units' scores+exp (PV deferred) ----------
            with tc.tile_pool(name="ps", bufs=2, space="PSUM") as psq:
                for hc in range(2):
                    ps = psq.tile([128, TQ], F32, tag="ps")
                    for e in range(2):
                        nc.tensor.matmul(
                            ps[:],
                            w_sb["q"][:, e * E + hc * 128: e * E + (hc + 1) * 128],
                            xq_sb[e][:, :],
                            start=(e == 0),
                            stop=(e == 1),
                        )
                    nc.vector.tensor_scalar_add(
                        qT[hc][:, :], ps[:], bq_sb[:, hc:hc + 1]
                    )
                for n in range(4):
                    for hc in range(2):
                        ps = psq.tile([128, 512], F32, tag="ps")
                        for e in range(2):
                            nc.tensor.matmul(
                                ps[:],
                                w_sb["k"][:, e * E + hc * 128: e * E + (hc + 1) * 128],
                                xk_sb[e][:, n * 512:(n + 1) * 512],
                                start=(e == 0),
                                stop=(e == 1),
                            )
                        # bk dropped: softmax(S + const-per-row) is
                        # invariant, and (q+bq).bk is constant across
                        # keys -> pure copy.
                        nc.vector.tensor_copy(
                            kT[hc][:, n * 512:(n + 1) * 512], ps[:]
                        )
                    for t in range(4 * n, 4 * n + 4):
                        ps = psq.tile([128, E], F32, tag="ps")
                        for e in range(2):
                            nc.tensor.matmul(
                                ps[:],
                                xv_sb[e][:, t * 128:(t + 1) * 128],
                                w_sb["v"][:, e * E:(e + 1) * E],
                                start=(e == 0),
                                stop=(e == 1),
                            )
                        vs = vsp.tile([128, E], BF16, tag="vstage")
                        # bv folds into the output bias (sum of softmax
                        # weights is 1): out += bv @ Wp, added at the
                        # tail -> pure copy.
                        nc.vector.tensor_copy(vs[:], ps[:])
                        nc.sync.dma_start(
                            out=v_buf[:, t * VW:(t + 1) * VW].rearrange(
                                "p (h w) -> p h w", h=H
                            )[:, :, 0:D],
                            in_=vs[:].rearrange("p (h d) -> p h d", h=H),
                        )
                    # attention units whose kT chunks are now projected:
                    # pass-0 cells with g <= 4n+3
                    while cursor[0] <= (4 * n + 4) * 4 - CPU:
                        emit_unit()

            # ---------- PV accumulators take over banks 6-7 ----------
            onat_t = {}
            osb_t = {}

            def finalize(p):
                """normalize + transpose O for pass p (proj at tail)."""
                poA, poB = po_tiles[p]
                onat = onatp.tile([128, TQ], BF16, name=f"onat{p}", tag="onat")
                osb = osbp.tile([128, TQ], BF16, name=f"osb{p}", tag="osb")
                rz = rzp.tile([128, 16], F32, name=f"rz{p}", tag="rz")
                onat_t[p], osb_t[p] = onat, osb
                for idx, po in ((0, poA), (1, poB)):
                    zv = po[:].rearrange("p (b m w) -> p b m w", b=2, m=4)[
                        :, :, :, D:D + 1
                    ]
                    rzo = rz[:, idx * 8:(idx + 1) * 8].rearrange(
                        "p (b m) -> p b m", b=2
                    ).unsqueeze(3)
                    nc.vector.reciprocal(rzo, zv)
                for u in range(4):
                    po = poA if u < 2 else poB
                    uu = u % 2
                    idx = u // 2
                    pin = po[:].rearrange("p (mm w) -> p mm w", w=33)[
                        :, uu * 4: uu * 4 + 4, 0:D
                    ]
                    rzb = rz[
                        :, idx * 8 + uu * 4: idx * 8 + uu * 4 + 4
                    ].unsqueeze(2).to_broadcast((128, 4, D))
                    pout = onat[:].rearrange(
                        "p (m b w) -> p m b w", m=4, b=4
                    )[:, :, u:u + 1, :]
                    nc.vector.tensor_tensor(pout, pin, rzb, ALU.mult)
                for m in range(4):
                    eng = nc.sync if m % 2 == 0 else nc.scalar
                    eng.dma_start_transpose(
                        osb[:, m * 128:(m + 1) * 128],
                        onat[:, m * 128:(m + 1) * 128],
                    )

            with tc.tile_pool(name="po", bufs=2, space="PSUM") as pop:
                po_tiles[0] = (
                    pop.tile([128, 264], F32, name="poA0", tag="po"),
                    pop.tile([128, 264], F32, name="poB0", tag="po"),
                )
                pv_enabled[0] = True
                flush_pv(keep=CPU)
                # emit remaining pass-0 cells (units may straddle into
                # pass 1; their pass-1 PVs wait in pv_pending)
                npass0_cells = NTK * 4
                while cursor[0] < npass0_cells:
                    emit_unit()
                while any(d[0] == 0 for d in pv_pending):
                    emit_pv_cell(pv_pending.pop(0))
                finalize(0)
                po_tiles[1] = (
                    pop.tile([128, 264], F32, name="poA1", tag="po"),
                    pop.tile([128, 264], F32, name="poB1", tag="po"),
                )
                flush_pv(keep=CPU)
                while emit_unit():
                    pass
                flush_pv()
                finalize(1)

                # ---------- tail: Wp projection + bias + out DMA ----------
                pjt = [
                    pop.tile([128, 2 * E], F32, name=f"pjt{i}", tag="po")
                    for i in range(2)
                ]
                # from the st pool: its slots are dead at the tail (the
                # po pool's 2 slots still hold the live pjt tiles)
                bbp = stp.tile([128, E], F32, name="bbp", tag="st")
                for e in range(2):
                    nc.tensor.matmul(
                        bbp[:],
                        bv_col[:, e:e + 1].to_broadcast((128, 128)),
                        w_sb["p"][:, e * E:(e + 1) * E],
                        start=(e == 0),
                        stop=(e == 1),
                    )
                bb_sb = pp.tile([128, E], F32, name="bb_sb")
                nc.vector.tensor_tensor(bb_sb[:], bbp[:], bp_sb[:], ALU.add)
                for m in range(4):
                    for p in range(NPASS):
                        nc.tensor.matmul(
                            pjt[m // 2][:, (m % 2) * E:(m % 2 + 1) * E],
                            osb_t[p][:, m * 128:(m + 1) * 128],
                            w_sb["p"][:, p * E:(p + 1) * E],
                            start=(p == 0 and m % 2 == 0),
                            stop=(p == NPASS - 1 and m % 2 == 1),
                            skip_group_check=True,
                        )
                for m in range(TQ // 128):
                    ob = outp.tile([128, E], F32, tag="outsb")
                    nc.vector.tensor_tensor(
                        ob[:], pjt[m // 2][:, (m % 2) * E:(m % 2 + 1) * E],
                        bb_sb[:], ALU.add,
                    )
                    eng = nc.sync if m % 2 == 0 else nc.scalar
                    eng.dma_start(
                        out=out[m * 128:(m + 1) * 128, :], in_=ob[:]
                    )

    return nc


def get_graph():
    global _GRAPH
    if _GRAPH is None:
        nc = _build_graph()
        nc.compile()
        _GRAPH = nc
    return _GRAPH


def make_in_maps(query, key_, value, Wq, bq, Wk, bk, Wv, bv, Wp, bp):
    query = np.asarray(query, np.float32)
    key_ = np.asarray(key_, np.float32)
    value = np.asarray(value, np.float32)
    Wq, Wk, Wv, Wp = (np.asarray(w, np.float32) for w in (Wq, Wk, Wv, Wp))
    bq, bk, bv, bp = (np.asarray(b_, np.float32) for b_ in (bq, bk, bv, bp))

    wq_b = np.ascontiguousarray(Wq).astype(BF)
    wk_b = np.ascontiguousarray(Wk).astype(BF)
    wv_b = np.ascontiguousarray(Wv).astype(BF)
    wp_b = np.ascontiguousarray(Wp).astype(BF)
    xt = {}
    for b in range(B):
        xt[("q", b)] = np.ascontiguousarray(query[:, b, :].T).astype(BF)
        xt[("k", b)] = np.ascontiguousarray(key_[:, b, :].T).astype(BF)
        xt[("v", b)] = np.ascontiguousarray(value[:, b, :].T).astype(BF)

    in_maps = []
    for c in range(NCORES):
        b = c // SP
        p = c % SP
        m = {
            "xqt": np.ascontiguousarray(xt[("q", b)][:, p * TQ:(p + 1) * TQ]),
            "xkt": xt[("k", b)],
            "xvt": xt[("v", b)],
            "wq": wq_b,
            "wk": wk_b,
            "wv": wv_b,
            "wp": wp_b,
            "bq": bq.reshape(1, E).copy(),
            "bk": bk.reshape(1, E).copy(),
            "bv": bv.reshape(1, E).copy(),
            "bp": bp.reshape(1, E).copy(),
        }
        in_maps.append(m)
    return in_maps


def assemble(results):
    out_full = np.empty((L, B, E), np.float32)
    for c in range(NCORES):
        b = c // SP
        p = c % SP
        out_full[p * TQ:(p + 1) * TQ, b, :] = results[c]["out"]
    return out_full


def run(inputs, trace=False, **kw):
    nc = get_graph()
    in_maps = make_in_maps(**inputs)
    res = run_bass_kernel_spmd(
        nc, in_maps, core_ids=list(range(NCORES)), trace=trace, **kw
    )
    return res


def kernel(**inputs):
    res = run(inputs, trace=False)
    return assemble(res.results)


# revision 22
# speedup vs baseline: 1.0894x; 1.0066x over previous
"""Distributed attention kernel for 8 TRN2 NeuronCores.

Problem: L=2048, B=2, E=256, H=8 heads, D=32 head-dim, fp32.

Sharding: DP2 over batch x sequence-parallel-4 over query positions.
Core c handles batch c//4, query rows [512*(c%4), 512*(c%4+1)), ALL 8
heads. k/v projections are redundantly computed per batch group (cheap)
and NO collective is needed: each core owns a disjoint output block.

Per-core pipeline (v4 -- cell units, ping-pong score tiles, dual exp):
  - The score work is 128 cells (pass, tk-chunk g, head h) of
    [K=32 d, M=128 tk, N=512 tq]. Cells run THREE at a time as one
    "unit": 3 concurrent PE matmuls on distinct 32-row bands
    (tile_position row tiling), each filling its own PSUM bank (a bank
    shared by concurrently-executing row-tiled matmuls hangs the
    device -- HW-verified).
  - TWO 3-bank score tiles ping-pong between units. Separate pool
    tiles are required: the Tile dep tracker is coarse-grained, so a
    shared tile serializes unit i+1's scores behind unit i's exp read
    (measured +0.7us/unit).
  - softmax exp runs on TWO engines in parallel: ScalarE exact exp via
    the ACT LUT; VectorE a Schraudolph exp2 (one fused mult+add
    tensor_scalar emitting the bf16 BIT PATTERN as int16, ~1.8% rms
    error, softmax-normalized). A minority of units take the DVE path
    so the output error stays ~1.3% (budget 2e-2).
  - PV is software-pipelined one unit behind (the PE is in-order; a PV
    waiting on exp would head-of-line-block the next scores), and
    deferred entirely while the q/k/v projections own the last two
    PSUM banks -- pool lifetimes let the projection psum, the PV
    accumulators, and the final projection accumulators share banks
    6-7 in sequence.
  - PV uses P.T chunks as STATIONARY and [v|1] as moving so O lands in
    natural [tq, d] orientation with the softmax denominator Z as a
    free per-partition column; xbar DMA transposes produce O.T, and
    the Wp projection runs at the tail with the per-head 1/Z folded in
    beforehand (reciprocal + broadcast multiply on DVE).
"""

import math
import os
import sys

import numpy as np

for _p in ("/opt/trn_rl_repo",):
    if _p not in sys.path and os.path.isdir(_p):
        sys.path.insert(0, _p)

import ml_dtypes

import concourse.bass as bass
import concourse.bacc as bacc
import concourse.mybir as mybir
import concourse.tile as tile
from concourse.bass_utils import run_bass_kernel_spmd

dt = mybir.dt
F32 = dt.float32
BF16 = dt.bfloat16
I16 = dt.int16
AF = mybir.ActivationFunctionType
ALU = mybir.AluOpType
BF = ml_dtypes.bfloat16

L, B, E, H, D = 2048, 2, 256, 8, 32
SCALE = float(D) ** -0.5
NCORES = 8
SP = 4            # sequence-parallel ways
TQ = L // SP      # 512 query rows per core
NTK = L // 128    # 16 tk chunks
VW = H * (D + 1)  # v_buf cols per tk chunk: 8x [v_h | 1] = 264
NPASS = 2         # head passes (4 heads each)
CPU = 3           # cells per unit

# Schraudolph exp2: bf16 bits of exp(s*SCALE) ~= int16(s*C1 + C2).
C1 = 128.0 * SCALE * math.log2(math.e)
C2 = 128.0 * (127.0 - 0.0434) + 0.5

_GRAPH = None


def _build_graph():
    nc = bacc.Bacc(
        "TRN2",
        target_bir_lowering=False,
        debug=False,
        enable_asserts=False,
        num_devices=NCORES,
    )

    xqt = nc.declare_dram_parameter("xqt", [E, TQ], BF16, isOutput=False).ap()
    xkt = nc.declare_dram_parameter("xkt", [E, L], BF16, isOutput=False).ap()
    xvt = nc.declare_dram_parameter("xvt", [E, L], BF16, isOutput=False).ap()
    wq = nc.declare_dram_parameter("wq", [E, E], BF16, isOutput=False).ap()
    wk = nc.declare_dram_parameter("wk", [E, E], BF16, isOutput=False).ap()
    wv = nc.declare_dram_parameter("wv", [E, E], BF16, isOutput=False).ap()
    wp = nc.declare_dram_parameter("wp", [E, E], BF16, isOutput=False).ap()
    bq = nc.declare_dram_parameter("bq", [1, E], F32, isOutput=False).ap()
    bk = nc.declare_dram_parameter("bk", [1, E], F32, isOutput=False).ap()
    bv = nc.declare_dram_parameter("bv", [1, E], F32, isOutput=False).ap()
    bp = nc.declare_dram_parameter("bp", [1, E], F32, isOutput=False).ap()
    out = nc.declare_dram_parameter("out", [TQ, E], F32, isOutput=True).ap()

    with tile.TileContext(nc) as tc:
        with (
            tc.tile_pool(name="persist", bufs=1) as pp,
            tc.tile_pool(name="pt", bufs=26) as ptp,
            tc.tile_pool(name="osb", bufs=2) as osbp,
            tc.tile_pool(name="onat", bufs=2) as onatp,
            tc.tile_pool(name="rz", bufs=2) as rzp,
            tc.tile_pool(name="vstage", bufs=4) as vsp,
            tc.tile_pool(name="outsb", bufs=2) as outp,
            tc.tile_pool(name="st", bufs=2, space="PSUM") as stp,
        ):
            # ---------- phase 0: loads ----------
            # weights: tile [128, 2E]; slice e covers W rows [128e, 128e+128)
            w_sb = {}

            def load_w(name, wsrc, eng):
                t = pp.tile([128, 2 * E], BF16, name=f"w{name}", tag=f"w{name}")
                eng.dma_start(
                    out=t[:].rearrange("p (e n) -> p e n", e=2),
                    in_=wsrc.rearrange("(e p) n -> p e n", p=128),
                )
                w_sb[name] = t

            # scalar queue: q-projection inputs first; sync queue: wk +
            # half of xk. v inputs and remaining weights come later.
            load_w("q", wq, nc.scalar)
            load_w("k", wk, nc.sync)
            xq_sb = []
            for e in range(2):
                t = pp.tile([128, TQ], BF16, name=f"xqt{e}", tag=f"xqt{e}")
                nc.scalar.dma_start(out=t[:], in_=xqt[e * 128:(e + 1) * 128, :])
                xq_sb.append(t)
            xk_sb = [
                pp.tile([128, L], BF16, name=f"xkt{e}", tag=f"xkt{e}")
                for e in range(2)
            ]
            for n in range(4):
                for e in range(2):
                    eng = nc.scalar if e == 0 else nc.sync
                    eng.dma_start(
                        out=xk_sb[e][:, n * 512:(n + 1) * 512],
                        in_=xkt[e * 128:(e + 1) * 128, n * 512:(n + 1) * 512],
                    )
            bq_sb = pp.tile([128, 2], F32)
            nc.gpsimd.dma_start(
                out=bq_sb[:], in_=bq.rearrange("a (c p) -> p (a c)", p=128)
            )
            bv_col = pp.tile([128, 2], BF16)
            nc.gpsimd.dma_start(
                out=bv_col[:], in_=bv.rearrange("a (c p) -> p (a c)", p=128)
            )
            bp_sb = pp.tile([128, E], F32)
            nc.gpsimd.dma_start(out=bp_sb[:], in_=bp.to_broadcast((128, E)))
            load_w("v", wv, nc.gpsimd)
            load_w("p", wp, nc.gpsimd)
            xv_sb = [
                pp.tile([128, L], BF16, name=f"xvt{e}", tag=f"xvt{e}")
                for e in range(2)
            ]
            for n in range(4):
                for e in range(2):
                    eng = nc.scalar if e == 0 else nc.sync
                    eng.dma_start(
                        out=xv_sb[e][:, n * 512:(n + 1) * 512],
                        in_=xvt[e * 128:(e + 1) * 128, n * 512:(n + 1) * 512],
                    )

            # warm the exp ACT table AFTER the load triggers are on the
            # scalar queue (the ~2.7us table load must not delay them)
            warm = pp.tile([1, 16], F32)
            nc.vector.memset(warm[:], 0.0)
            nc.scalar.activation(warm[:], warm[:], AF.Exp)

            # ---------- persistent SBUF state ----------
            # kT[hc]: [128 = 4 heads x 32 d (bands 0/32/64/96), 2048 tk]
            kT = [pp.tile([128, L], BF16, name=f"kT{hc}", tag=f"kT{hc}")
                  for hc in range(2)]
            qT = [pp.tile([128, TQ], BF16, name=f"qT{hc}", tag=f"qT{hc}")
                  for hc in range(2)]
            v_buf = pp.tile([128, NTK * VW], BF16)
            nc.gpsimd.memset(v_buf[:], 1.0)

            # ping-pong score tiles: 3 banks each (bank r <-> the r-th
            # row band used by the unit)
            st_ab = [
                stp.tile([128, CPU * 512], F32, name=f"st{i}", tag="st")
                for i in range(2)
            ]

            # ---------- cell/unit machinery ----------
            # cell = (pass, g, h): scores for head 4p+h over tk chunk g,
            # all 512 tq. Units take 3 consecutive cells (distinct h mod
            # 4 -> distinct PE row bands).
            cells = [(p, g, h) for p in range(NPASS) for g in range(NTK)
                     for h in range(4)]
            cursor = [0]        # next cell index
            unit_no = [0]
            pv_pending = []     # descs awaiting PV emission
            pv_enabled = [False]
            po_tiles = {}

            def emit_pv_cell(desc):
                p, g, h, pt, r = desc
                poA, poB = po_tiles[p]
                po = poA if h < 2 else poB
                uu = h % 2
                hh = 4 * p + h
                for m in range(4):
                    nc.tensor.matmul(
                        po[:, uu * 132 + m * 33: uu * 132 + m * 33 + 33],
                        pt[:, r * 512 + m * 128: r * 512 + (m + 1) * 128],
                        v_buf[:, g * VW + hh * (D + 1): g * VW + (hh + 1) * (D + 1)],
                        start=(g == 0 and uu == 0 and m == 0),
                        stop=(g == NTK - 1 and uu == 1 and m == 3),
                        skip_group_check=True,
                    )

            def flush_pv(keep=0):
                while len(pv_pending) > keep:
                    emit_pv_cell(pv_pending.pop(0))

            def emit_unit():
                """scores + exp for the next <=3 cells; queues their PV."""
                lo = cursor[0]
                hi = min(lo + CPU, len(cells))
                if lo >= hi:
                    return False
                cursor[0] = hi
                q = unit_no[0]
                unit_no[0] += 1
                st = st_ab[q % 2]
                ncell = hi - lo
                for r in range(ncell):
                    p, g, h = cells[lo + r]
                    nc.tensor.matmul(
                        st[:, r * 512:(r + 1) * 512],
                        kT[p][32 * h:32 * h + D, g * 128:(g + 1) * 128],
                        qT[p][32 * h:32 * h + D, :],
                        start=True,
                        stop=True,
                        tile_position=(32 * h, 0),
                    )
                pt = ptp.tile([128, CPU * 512], BF16, tag="pt")
                # exp split WITHIN the unit: ScalarE takes the first two
                # cells (exact exp), DVE the third (Schraudolph). Both
                # run concurrently, so the unit's exp latency is the
                # ScalarE instruction (~1.1us), which fits under the
                # two-unit PE budget of the st-tile ping-pong chain.
                ws = min(2, ncell) * 512
                nc.scalar.activation(
                    pt[:, 0:ws], st[:, 0:ws], AF.Exp, scale=SCALE
                )
                if ncell == CPU:
                    nc.vector.tensor_scalar(
                        pt[:, ws:ws + 512].bitcast(I16), st[:, ws:ws + 512],
                        C1, C2, ALU.mult, ALU.add,
                    )
                for r in range(ncell):
                    p, g, h = cells[lo + r]
                    pv_pending.append((p, g, h, pt, r))
                if pv_enabled[0]:
                    flush_pv(keep=CPU)
                return True

            # ---------- projections (psum banks 6-7), interleaved with
            # the first attention units' scores+exp (PV deferred) ----------
            with tc.tile_pool(name="ps", bufs=2, space="PSUM") as psq:
                for hc in range(2):
                    ps = psq.tile([128, TQ], F32, tag="ps")
                    for e in range(2):
                        nc.tensor.matmul(
                            ps[:],
                            w_sb["q"][:, e * E + hc * 128: e * E + (hc + 1) * 128],
                            xq_sb[e][:, :],
                            start=(e == 0),
                            stop=(e == 1),
                        )
                    nc.vector.tensor_scalar_add(
                        qT[hc][:, :], ps[:], bq_sb[:, hc:hc + 1]
                    )
                for n in range(4):
                    for hc in range(2):
                        ps = psq.tile([128, 512], F32, tag="ps")
                        for e in range(2):
                            nc.tensor.matmul(
                                ps[:],
                                w_sb["k"][:, e * E + hc * 128: e * E + (hc + 1) * 128],
                                xk_sb[e][:, n * 512:(n + 1) * 512],
                                start=(e == 0),
                                stop=(e == 1),
                            )
                        # bk dropped: softmax(S + const-per-row) is
                        # invariant, and (q+bq).bk is constant across
                        # keys -> pure copy.
                        nc.vector.tensor_copy(
                            kT[hc][:, n * 512:(n + 1) * 512], ps[:]
                        )
                    for t in range(4 * n, 4 * n + 4):
                        ps = psq.tile([128, E], F32, tag="ps")
                        for e in range(2):
                            nc.tensor.matmul(
                                ps[:],
                                xv_sb[e][:, t * 128:(t + 1) * 128],
                                w_sb["v"][:, e * E:(e + 1) * E],
                                start=(e == 0),
                                stop=(e == 1),
                            )
                        vs = vsp.tile([128, E], BF16, tag="vstage")
                        # bv folds into the output bias (sum of softmax
                        # weights is 1): out += bv @ Wp, added at the
                        # tail -> pure copy.
                        nc.vector.tensor_copy(vs[:], ps[:])
                        nc.sync.dma_start(
                            out=v_buf[:, t * VW:(t + 1) * VW].rearrange(
                                "p (h w) -> p h w", h=H
                            )[:, :, 0:D],
                            in_=vs[:].rearrange("p (h d) -> p h d", h=H),
                        )
                    # attention units whose kT chunks are now projected:
                    # pass-0 cells with g <= 4n+3
                    while cursor[0] <= (4 * n + 4) * 4 - CPU:
                        emit_unit()

            # ---------- PV accumulators take over banks 6-7 ----------
            onat_t = {}
            osb_t = {}

            def finalize(p):
                """normalize + transpose O for pass p (proj at tail)."""
                poA, poB = po_tiles[p]
                onat = onatp.tile([128, TQ], BF16, name=f"onat{p}", tag="onat")
                osb = osbp.tile([128, TQ], BF16, name=f"osb{p}", tag="osb")
                rz = rzp.tile([128, 16], F32, name=f"rz{p}", tag="rz")
                onat_t[p], osb_t[p] = onat, osb
                for idx, po in ((0, poA), (1, poB)):
                    zv = po[:].rearrange("p (b m w) -> p b m w", b=2, m=4)[
                        :, :, :, D:D + 1
                    ]
                    rzo = rz[:, idx * 8:(idx + 1) * 8].rearrange(
                        "p (b m) -> p b m", b=2
                    ).unsqueeze(3)
                    nc.vector.reciprocal(rzo, zv)
                for m in range(4):
                    for idx, po in ((0, poA), (1, poB)):
                        # both uu of this po, m-th chunk: [128, 2, 32]
                        pin = po[:].rearrange(
                            "p (b mm w) -> p b mm w", b=2, w=33
                        )[:, :, m:m + 1, 0:D]
                        rzb = rz[:, idx * 8:(idx + 1) * 8].rearrange(
                            "p (b mm) -> p b mm", b=2
                        )[:, :, m:m + 1].unsqueeze(3).to_broadcast(
                            (128, 2, 1, D)
                        )
                        pout = onat[:].rearrange(
                            "p (mm b w) -> p mm b w", mm=4, b=4
                        )[:, m:m + 1, 2 * idx:2 * idx + 2, :]
                        nc.vector.tensor_tensor(pout, pin, rzb, ALU.mult)
                    eng = nc.sync if m % 2 == 0 else nc.scalar
                    eng.dma_start_transpose(
                        osb[:, m * 128:(m + 1) * 128],
                        onat[:, m * 128:(m + 1) * 128],
                    )

            with tc.tile_pool(name="po", bufs=2, space="PSUM") as pop:
                po_tiles[0] = (
                    pop.tile([128, 264], F32, name="poA0", tag="po"),
                    pop.tile([128, 264], F32, name="poB0", tag="po"),
                )
                pv_enabled[0] = True
                flush_pv(keep=CPU)
                # emit remaining pass-0 cells (units may straddle into
                # pass 1; their pass-1 PVs wait in pv_pending)
                npass0_cells = NTK * 4
                while cursor[0] < npass0_cells:
                    emit_unit()
                while any(d[0] == 0 for d in pv_pending):
                    emit_pv_cell(pv_pending.pop(0))
                finalize(0)
                po_tiles[1] = (
                    pop.tile([128, 264], F32, name="poA1", tag="po"),
                    pop.tile([128, 264], F32, name="poB1", tag="po"),
                )
                flush_pv(keep=CPU)
                while emit_unit():
                    pass
                flush_pv()
                finalize(1)

                # ---------- tail: Wp projection + bias + out DMA ----------
                pjt = [
                    pop.tile([128, 2 * E], F32, name=f"pjt{i}", tag="po")
                    for i in range(2)
                ]
                # from the st pool: its slots are dead at the tail (the
                # po pool's 2 slots still hold the live pjt tiles)
                bbp = stp.tile([128, E], F32, name="bbp", tag="st")
                for e in range(2):
                    nc.tensor.matmul(
                        bbp[:],
                        bv_col[:, e:e + 1].to_broadcast((128, 128)),
                        w_sb["p"][:, e * E:(e + 1) * E],
                        start=(e == 0),
                        stop=(e == 1),
                    )
                bb_sb = pp.tile([128, E], F32, name="bb_sb")
                nc.vector.tensor_tensor(bb_sb[:], bbp[:], bp_sb[:], ALU.add)
                for m in range(4):
                    for p in range(NPASS):
                        nc.tensor.matmul(
                            pjt[m // 2][:, (m % 2) * E:(m % 2 + 1) * E],
                            osb_t[p][:, m * 128:(m + 1) * 128],
                            w_sb["p"][:, p * E:(p + 1) * E],
                            start=(p == 0 and m % 2 == 0),
                            stop=(p == NPASS - 1 and m % 2 == 1),
                            skip_group_check=True,
                        )
                for m in range(TQ // 128):
                    ob = outp.tile([128, E], F32, tag="outsb")
                    nc.vector.tensor_tensor(
                        ob[:], pjt[m // 2][:, (m % 2) * E:(m % 2 + 1) * E],
                        bb_sb[:], ALU.add,
                    )
                    eng = nc.sync if m % 2 == 0 else nc.scalar
                    eng.dma_start(
                        out=out[m * 128:(m + 1) * 128, :], in_=ob[:]
                    )

    return nc


def get_graph():
    global _GRAPH
    if _GRAPH is None:
        nc = _build_graph()
        nc.compile()
        _GRAPH = nc
    return _GRAPH


def make_in_maps(query, key_, value, Wq, bq, Wk, bk, Wv, bv, Wp, bp):
    query = np.asarray(query, np.float32)
    key_ = np.asarray(key_, np.float32)
    value = np.asarray(value, np.float32)
    Wq, Wk, Wv, Wp = (np.asarray(w, np.float32) for w in (Wq, Wk, Wv, Wp))
    bq, bk, bv, bp = (np.asarray(b_, np.float32) for b_ in (bq, bk, bv, bp))

    wq_b = np.ascontiguousarray(Wq).astype(BF)
    wk_b = np.ascontiguousarray(Wk).astype(BF)
    wv_b = np.ascontiguousarray(Wv).astype(BF)
    wp_b = np.ascontiguousarray(Wp).astype(BF)
    xt = {}
    for b in range(B):
        xt[("q", b)] = np.ascontiguousarray(query[:, b, :].T).astype(BF)
        xt[("k", b)] = np.ascontiguousarray(key_[:, b, :].T).astype(BF)
        xt[("v", b)] = np.ascontiguousarray(value[:, b, :].T).astype(BF)

    in_maps = []
    for c in range(NCORES):
        b = c // SP
        p = c % SP
        m = {
            "xqt": np.ascontiguousarray(xt[("q", b)][:, p * TQ:(p + 1) * TQ]),
            "xkt": xt[("k", b)],
            "xvt": xt[("v", b)],
            "wq": wq_b,
            "wk": wk_b,
            "wv": wv_b,
            "wp": wp_b,
            "bq": bq.reshape(1, E).copy(),
            "bk": bk.reshape(1, E).copy(),
            "bv": bv.reshape(1, E).copy(),
            "bp": bp.reshape(1, E).copy(),
        }
        in_maps.append(m)
    return in_maps


def assemble(results):
    out_full = np.empty((L, B, E), np.float32)
    for c in range(NCORES):
        b = c // SP
        p = c % SP
        out_full[p * TQ:(p + 1) * TQ, b, :] = results[c]["out"]
    return out_full


def run(inputs, trace=False, **kw):
    nc = get_graph()
    in_maps = make_in_maps(**inputs)
    res = run_bass_kernel_spmd(
        nc, in_maps, core_ids=list(range(NCORES)), trace=trace, **kw
    )
    return res


def kernel(**inputs):
    res = run(inputs, trace=False)
    return assemble(res.results)


# revision 23
# speedup vs baseline: 1.1061x; 1.0153x over previous
"""Distributed attention kernel for 8 TRN2 NeuronCores.

Problem: L=2048, B=2, E=256, H=8 heads, D=32 head-dim, fp32.

Sharding: DP2 over batch x sequence-parallel-4 over query positions.
Core c handles batch c//4, query rows [512*(c%4), 512*(c%4+1)), ALL 8
heads. k/v projections are redundantly computed per batch group (cheap)
and NO collective is needed: each core owns a disjoint output block.

Per-core pipeline (v4 -- cell units, ping-pong score tiles, dual exp):
  - The score work is 128 cells (pass, tk-chunk g, head h) of
    [K=32 d, M=128 tk, N=512 tq]. Cells run THREE at a time as one
    "unit": 3 concurrent PE matmuls on distinct 32-row bands
    (tile_position row tiling), each filling its own PSUM bank (a bank
    shared by concurrently-executing row-tiled matmuls hangs the
    device -- HW-verified).
  - TWO 3-bank score tiles ping-pong between units. Separate pool
    tiles are required: the Tile dep tracker is coarse-grained, so a
    shared tile serializes unit i+1's scores behind unit i's exp read
    (measured +0.7us/unit).
  - softmax exp runs on TWO engines in parallel: ScalarE exact exp via
    the ACT LUT; VectorE a Schraudolph exp2 (one fused mult+add
    tensor_scalar emitting the bf16 BIT PATTERN as int16, ~1.8% rms
    error, softmax-normalized). A minority of units take the DVE path
    so the output error stays ~1.3% (budget 2e-2).
  - PV is software-pipelined one unit behind (the PE is in-order; a PV
    waiting on exp would head-of-line-block the next scores), and
    deferred entirely while the q/k/v projections own the last two
    PSUM banks -- pool lifetimes let the projection psum, the PV
    accumulators, and the final projection accumulators share banks
    6-7 in sequence.
  - PV uses P.T chunks as STATIONARY and [v|1] as moving so O lands in
    natural [tq, d] orientation with the softmax denominator Z as a
    free per-partition column; xbar DMA transposes produce O.T, and
    the Wp projection runs at the tail with the per-head 1/Z folded in
    beforehand (reciprocal + broadcast multiply on DVE).
"""

import math
import os
import sys

import numpy as np

for _p in ("/opt/trn_rl_repo",):
    if _p not in sys.path and os.path.isdir(_p):
        sys.path.insert(0, _p)

import ml_dtypes

import concourse.bass as bass
import concourse.bacc as bacc
import concourse.mybir as mybir
import concourse.tile as tile
from concourse.bass_utils import run_bass_kernel_spmd

dt = mybir.dt
F32 = dt.float32
BF16 = dt.bfloat16
I16 = dt.int16
AF = mybir.ActivationFunctionType
ALU = mybir.AluOpType
BF = ml_dtypes.bfloat16

L, B, E, H, D = 2048, 2, 256, 8, 32
SCALE = float(D) ** -0.5
NCORES = 8
SP = 4            # sequence-parallel ways
TQ = L // SP      # 512 query rows per core
NTK = L // 128    # 16 tk chunks
VW = H * (D + 1)  # v_buf cols per tk chunk: 8x [v_h | 1] = 264
NPASS = 2         # head passes (4 heads each)
CPU = 3           # cells per unit

# Schraudolph exp2: bf16 bits of exp(s*SCALE) ~= int16(s*C1 + C2).
C1 = 128.0 * SCALE * math.log2(math.e)
C2 = 128.0 * (127.0 - 0.0434) + 0.5

_GRAPH = None


def _build_graph():
    nc = bacc.Bacc(
        "TRN2",
        target_bir_lowering=False,
        debug=False,
        enable_asserts=False,
        num_devices=NCORES,
    )

    xqt = nc.declare_dram_parameter("xqt", [E, TQ], BF16, isOutput=False).ap()
    xkt = nc.declare_dram_parameter("xkt", [E, L], BF16, isOutput=False).ap()
    xvt = nc.declare_dram_parameter("xvt", [E, L], BF16, isOutput=False).ap()
    wq = nc.declare_dram_parameter("wq", [E, E], BF16, isOutput=False).ap()
    wk = nc.declare_dram_parameter("wk", [E, E], BF16, isOutput=False).ap()
    wv = nc.declare_dram_parameter("wv", [E, E], BF16, isOutput=False).ap()
    wp = nc.declare_dram_parameter("wp", [E, E], BF16, isOutput=False).ap()
    bq = nc.declare_dram_parameter("bq", [1, E], F32, isOutput=False).ap()
    bk = nc.declare_dram_parameter("bk", [1, E], F32, isOutput=False).ap()
    bv = nc.declare_dram_parameter("bv", [1, E], F32, isOutput=False).ap()
    bp = nc.declare_dram_parameter("bp", [1, E], F32, isOutput=False).ap()
    out = nc.declare_dram_parameter("out", [TQ, E], F32, isOutput=True).ap()

    with tile.TileContext(nc) as tc:
        with (
            tc.tile_pool(name="persist", bufs=1) as pp,
            tc.tile_pool(name="pt", bufs=26) as ptp,
            tc.tile_pool(name="osb", bufs=2) as osbp,
            tc.tile_pool(name="onat", bufs=2) as onatp,
            tc.tile_pool(name="rz", bufs=2) as rzp,
            tc.tile_pool(name="vstage", bufs=4) as vsp,
            tc.tile_pool(name="outsb", bufs=2) as outp,
            tc.tile_pool(name="st", bufs=2, space="PSUM") as stp,
        ):
            # ---------- phase 0: loads ----------
            # weights: tile [128, 2E]; slice e covers W rows [128e, 128e+128)
            w_sb = {}

            def load_w(name, wsrc, eng):
                t = pp.tile([128, 2 * E], BF16, name=f"w{name}", tag=f"w{name}")
                eng.dma_start(
                    out=t[:].rearrange("p (e n) -> p e n", e=2),
                    in_=wsrc.rearrange("(e p) n -> p e n", p=128),
                )
                w_sb[name] = t

            # scalar queue: q-projection inputs first; sync queue: wk +
            # half of xk. v inputs and remaining weights come later.
            load_w("q", wq, nc.scalar)
            xq_sb = []
            for e in range(2):
                t = pp.tile([128, TQ], BF16, name=f"xqt{e}", tag=f"xqt{e}")
                eng = nc.scalar if e == 0 else nc.sync
                eng.dma_start(out=t[:], in_=xqt[e * 128:(e + 1) * 128, :])
                xq_sb.append(t)
            load_w("k", wk, nc.sync)
            xk_sb = [
                pp.tile([128, L], BF16, name=f"xkt{e}", tag=f"xkt{e}")
                for e in range(2)
            ]
            for n in range(4):
                for e in range(2):
                    eng = nc.scalar if e == 0 else nc.sync
                    eng.dma_start(
                        out=xk_sb[e][:, n * 512:(n + 1) * 512],
                        in_=xkt[e * 128:(e + 1) * 128, n * 512:(n + 1) * 512],
                    )
            bq_sb = pp.tile([128, 2], F32)
            nc.gpsimd.dma_start(
                out=bq_sb[:], in_=bq.rearrange("a (c p) -> p (a c)", p=128)
            )
            bv_col = pp.tile([128, 2], BF16)
            nc.gpsimd.dma_start(
                out=bv_col[:], in_=bv.rearrange("a (c p) -> p (a c)", p=128)
            )
            bp_sb = pp.tile([128, E], F32)
            nc.gpsimd.dma_start(out=bp_sb[:], in_=bp.to_broadcast((128, E)))
            load_w("v", wv, nc.gpsimd)
            load_w("p", wp, nc.gpsimd)
            # xv rides the gpsimd SWDGE queue, keeping the two HWDGE
            # queues free for the latency-critical q/k inputs
            xv_sb = [
                pp.tile([128, L], BF16, name=f"xvt{e}", tag=f"xvt{e}")
                for e in range(2)
            ]
            for n in range(4):
                for e in range(2):
                    nc.gpsimd.dma_start(
                        out=xv_sb[e][:, n * 512:(n + 1) * 512],
                        in_=xvt[e * 128:(e + 1) * 128, n * 512:(n + 1) * 512],
                    )

            # warm the exp ACT table AFTER the load triggers are on the
            # scalar queue (the ~2.7us table load must not delay them)
            warm = pp.tile([1, 16], F32)
            nc.vector.memset(warm[:], 0.0)
            nc.scalar.activation(warm[:], warm[:], AF.Exp)

            # ---------- persistent SBUF state ----------
            # kT[hc]: [128 = 4 heads x 32 d (bands 0/32/64/96), 2048 tk]
            kT = [pp.tile([128, L], BF16, name=f"kT{hc}", tag=f"kT{hc}")
                  for hc in range(2)]
            qT = [pp.tile([128, TQ], BF16, name=f"qT{hc}", tag=f"qT{hc}")
                  for hc in range(2)]
            v_buf = pp.tile([128, NTK * VW], BF16)
            nc.gpsimd.memset(v_buf[:], 1.0)

            # ping-pong score tiles: 3 banks each (bank r <-> the r-th
            # row band used by the unit)
            st_ab = [
                stp.tile([128, CPU * 512], F32, name=f"st{i}", tag="st")
                for i in range(2)
            ]

            # ---------- cell/unit machinery ----------
            # cell = (pass, g, h): scores for head 4p+h over tk chunk g,
            # all 512 tq. Units take 3 consecutive cells (distinct h mod
            # 4 -> distinct PE row bands).
            cells = [(p, g, h) for p in range(NPASS) for g in range(NTK)
                     for h in range(4)]
            cursor = [0]        # next cell index
            unit_no = [0]
            pv_pending = []     # descs awaiting PV emission
            pv_enabled = [False]
            po_tiles = {}

            def emit_pv_cell(desc):
                p, g, h, pt, r = desc
                poA, poB = po_tiles[p]
                po = poA if h < 2 else poB
                uu = h % 2
                hh = 4 * p + h
                for m in range(4):
                    nc.tensor.matmul(
                        po[:, uu * 132 + m * 33: uu * 132 + m * 33 + 33],
                        pt[:, r * 512 + m * 128: r * 512 + (m + 1) * 128],
                        v_buf[:, g * VW + hh * (D + 1): g * VW + (hh + 1) * (D + 1)],
                        start=(g == 0 and uu == 0 and m == 0),
                        stop=(g == NTK - 1 and uu == 1 and m == 3),
                        skip_group_check=True,
                    )

            def flush_pv(keep=0):
                while len(pv_pending) > keep:
                    emit_pv_cell(pv_pending.pop(0))

            def emit_unit():
                """scores + exp for the next <=3 cells; queues their PV."""
                lo = cursor[0]
                hi = min(lo + CPU, len(cells))
                if lo >= hi:
                    return False
                cursor[0] = hi
                q = unit_no[0]
                unit_no[0] += 1
                st = st_ab[q % 2]
                ncell = hi - lo
                for r in range(ncell):
                    p, g, h = cells[lo + r]
                    nc.tensor.matmul(
                        st[:, r * 512:(r + 1) * 512],
                        kT[p][32 * h:32 * h + D, g * 128:(g + 1) * 128],
                        qT[p][32 * h:32 * h + D, :],
                        start=True,
                        stop=True,
                        tile_position=(32 * h, 0),
                    )
                pt = ptp.tile([128, CPU * 512], BF16, tag="pt")
                # exp split WITHIN the unit: ScalarE takes the first two
                # cells (exact exp), DVE the third (Schraudolph). Both
                # run concurrently, so the unit's exp latency is the
                # ScalarE instruction (~1.1us), which fits under the
                # two-unit PE budget of the st-tile ping-pong chain.
                ws = min(2, ncell) * 512
                nc.scalar.activation(
                    pt[:, 0:ws], st[:, 0:ws], AF.Exp, scale=SCALE
                )
                if ncell == CPU:
                    nc.vector.tensor_scalar(
                        pt[:, ws:ws + 512].bitcast(I16), st[:, ws:ws + 512],
                        C1, C2, ALU.mult, ALU.add,
                    )
                for r in range(ncell):
                    p, g, h = cells[lo + r]
                    pv_pending.append((p, g, h, pt, r))
                if pv_enabled[0]:
                    flush_pv(keep=2 * CPU)
                return True

            # ---------- projections (psum banks 6-7), interleaved with
            # the first attention units' scores+exp (PV deferred) ----------
            with tc.tile_pool(name="ps", bufs=2, space="PSUM") as psq:
                for hc in range(2):
                    ps = psq.tile([128, TQ], F32, tag="ps")
                    for e in range(2):
                        nc.tensor.matmul(
                            ps[:],
                            w_sb["q"][:, e * E + hc * 128: e * E + (hc + 1) * 128],
                            xq_sb[e][:, :],
                            start=(e == 0),
                            stop=(e == 1),
                        )
                    nc.vector.tensor_scalar_add(
                        qT[hc][:, :], ps[:], bq_sb[:, hc:hc + 1]
                    )
                for n in range(4):
                    for hc in range(2):
                        ps = psq.tile([128, 512], F32, tag="ps")
                        for e in range(2):
                            nc.tensor.matmul(
                                ps[:],
                                w_sb["k"][:, e * E + hc * 128: e * E + (hc + 1) * 128],
                                xk_sb[e][:, n * 512:(n + 1) * 512],
                                start=(e == 0),
                                stop=(e == 1),
                            )
                        # bk dropped: softmax(S + const-per-row) is
                        # invariant, and (q+bq).bk is constant across
                        # keys -> pure copy.
                        nc.vector.tensor_copy(
                            kT[hc][:, n * 512:(n + 1) * 512], ps[:]
                        )
                    for t in range(4 * n, 4 * n + 4):
                        ps = psq.tile([128, E], F32, tag="ps")
                        for e in range(2):
                            nc.tensor.matmul(
                                ps[:],
                                xv_sb[e][:, t * 128:(t + 1) * 128],
                                w_sb["v"][:, e * E:(e + 1) * E],
                                start=(e == 0),
                                stop=(e == 1),
                            )
                        vs = vsp.tile([128, E], BF16, tag="vstage")
                        # bv folds into the output bias (sum of softmax
                        # weights is 1): out += bv @ Wp, added at the
                        # tail -> pure copy.
                        nc.vector.tensor_copy(vs[:], ps[:])
                        nc.sync.dma_start(
                            out=v_buf[:, t * VW:(t + 1) * VW].rearrange(
                                "p (h w) -> p h w", h=H
                            )[:, :, 0:D],
                            in_=vs[:].rearrange("p (h d) -> p h d", h=H),
                        )
                    # attention units whose kT chunks are now projected:
                    # pass-0 cells with g <= 4n+3
                    while cursor[0] <= (4 * n + 4) * 4 - CPU:
                        emit_unit()

            # ---------- PV accumulators take over banks 6-7 ----------
            onat_t = {}
            osb_t = {}

            def finalize(p):
                """normalize + transpose O for pass p (proj at tail)."""
                poA, poB = po_tiles[p]
                onat = onatp.tile([128, TQ], BF16, name=f"onat{p}", tag="onat")
                osb = osbp.tile([128, TQ], BF16, name=f"osb{p}", tag="osb")
                rz = rzp.tile([128, 16], F32, name=f"rz{p}", tag="rz")
                onat_t[p], osb_t[p] = onat, osb
                for idx, po in ((0, poA), (1, poB)):
                    zv = po[:].rearrange("p (b m w) -> p b m w", b=2, m=4)[
                        :, :, :, D:D + 1
                    ]
                    rzo = rz[:, idx * 8:(idx + 1) * 8].rearrange(
                        "p (b m) -> p b m", b=2
                    ).unsqueeze(3)
                    nc.vector.reciprocal(rzo, zv)
                for m in range(4):
                    for idx, po in ((0, poA), (1, poB)):
                        # both uu of this po, m-th chunk: [128, 2, 32]
                        pin = po[:].rearrange(
                            "p (b mm w) -> p b mm w", b=2, w=33
                        )[:, :, m:m + 1, 0:D]
                        rzb = rz[:, idx * 8:(idx + 1) * 8].rearrange(
                            "p (b mm) -> p b mm", b=2
                        )[:, :, m:m + 1].unsqueeze(3).to_broadcast(
                            (128, 2, 1, D)
                        )
                        pout = onat[:].rearrange(
                            "p (mm b w) -> p mm b w", mm=4, b=4
                        )[:, m:m + 1, 2 * idx:2 * idx + 2, :]
                        nc.vector.tensor_tensor(pout, pin, rzb, ALU.mult)
                    eng = nc.sync if m % 2 == 0 else nc.scalar
                    eng.dma_start_transpose(
                        osb[:, m * 128:(m + 1) * 128],
                        onat[:, m * 128:(m + 1) * 128],
                    )

            with tc.tile_pool(name="po", bufs=2, space="PSUM") as pop:
                po_tiles[0] = (
                    pop.tile([128, 264], F32, name="poA0", tag="po"),
                    pop.tile([128, 264], F32, name="poB0", tag="po"),
                )
                pv_enabled[0] = True
                flush_pv(keep=CPU)
                # emit remaining pass-0 cells (units may straddle into
                # pass 1; their pass-1 PVs wait in pv_pending)
                npass0_cells = NTK * 4
                while cursor[0] < npass0_cells:
                    emit_unit()
                while any(d[0] == 0 for d in pv_pending):
                    emit_pv_cell(pv_pending.pop(0))
                finalize(0)
                po_tiles[1] = (
                    pop.tile([128, 264], F32, name="poA1", tag="po"),
                    pop.tile([128, 264], F32, name="poB1", tag="po"),
                )
                flush_pv(keep=CPU)
                while emit_unit():
                    pass
                flush_pv()
                finalize(1)

                # ---------- tail: Wp projection + bias + out DMA ----------
                pjt = [
                    pop.tile([128, 2 * E], F32, name=f"pjt{i}", tag="po")
                    for i in range(2)
                ]
                # from the st pool: its slots are dead at the tail (the
                # po pool's 2 slots still hold the live pjt tiles)
                bbp = stp.tile([128, E], F32, name="bbp", tag="st")
                for e in range(2):
                    nc.tensor.matmul(
                        bbp[:],
                        bv_col[:, e:e + 1].to_broadcast((128, 128)),
                        w_sb["p"][:, e * E:(e + 1) * E],
                        start=(e == 0),
                        stop=(e == 1),
                    )
                bb_sb = pp.tile([128, E], F32, name="bb_sb")
                nc.vector.tensor_tensor(bb_sb[:], bbp[:], bp_sb[:], ALU.add)
                for m in range(4):
                    for p in range(NPASS):
                        nc.tensor.matmul(
                            pjt[m // 2][:, (m % 2) * E:(m % 2 + 1) * E],
                            osb_t[p][:, m * 128:(m + 1) * 128],
                            w_sb["p"][:, p * E:(p + 1) * E],
                            start=(p == 0 and m % 2 == 0),
                            stop=(p == NPASS - 1 and m % 2 == 1),
                            skip_group_check=True,
                        )
                for m in range(TQ // 128):
                    ob = outp.tile([128, E], F32, tag="outsb")
                    nc.vector.tensor_tensor(
                        ob[:], pjt[m // 2][:, (m % 2) * E:(m % 2 + 1) * E],
                        bb_sb[:], ALU.add,
                    )
                    eng = nc.sync if m % 2 == 0 else nc.scalar
                    eng.dma_start(
                        out=out[m * 128:(m + 1) * 128, :], in_=ob[:]
                    )

    return nc


def get_graph():
    global _GRAPH
    if _GRAPH is None:
        nc = _build_graph()
        nc.compile()
        _GRAPH = nc
    return _GRAPH


def make_in_maps(query, key_, value, Wq, bq, Wk, bk, Wv, bv, Wp, bp):
    query = np.asarray(query, np.float32)
    key_ = np.asarray(key_, np.float32)
    value = np.asarray(value, np.float32)
    Wq, Wk, Wv, Wp = (np.asarray(w, np.float32) for w in (Wq, Wk, Wv, Wp))
    bq, bk, bv, bp = (np.asarray(b_, np.float32) for b_ in (bq, bk, bv, bp))

    wq_b = np.ascontiguousarray(Wq).astype(BF)
    wk_b = np.ascontiguousarray(Wk).astype(BF)
    wv_b = np.ascontiguousarray(Wv).astype(BF)
    wp_b = np.ascontiguousarray(Wp).astype(BF)
    xt = {}
    for b in range(B):
        xt[("q", b)] = np.ascontiguousarray(query[:, b, :].T).astype(BF)
        xt[("k", b)] = np.ascontiguousarray(key_[:, b, :].T).astype(BF)
        xt[("v", b)] = np.ascontiguousarray(value[:, b, :].T).astype(BF)

    in_maps = []
    for c in range(NCORES):
        b = c // SP
        p = c % SP
        m = {
            "xqt": np.ascontiguousarray(xt[("q", b)][:, p * TQ:(p + 1) * TQ]),
            "xkt": xt[("k", b)],
            "xvt": xt[("v", b)],
            "wq": wq_b,
            "wk": wk_b,
            "wv": wv_b,
            "wp": wp_b,
            "bq": bq.reshape(1, E).copy(),
            "bk": bk.reshape(1, E).copy(),
            "bv": bv.reshape(1, E).copy(),
            "bp": bp.reshape(1, E).copy(),
        }
        in_maps.append(m)
    return in_maps


def assemble(results):
    out_full = np.empty((L, B, E), np.float32)
    for c in range(NCORES):
        b = c // SP
        p = c % SP
        out_full[p * TQ:(p + 1) * TQ, b, :] = results[c]["out"]
    return out_full


def run(inputs, trace=False, **kw):
    nc = get_graph()
    in_maps = make_in_maps(**inputs)
    res = run_bass_kernel_spmd(
        nc, in_maps, core_ids=list(range(NCORES)), trace=trace, **kw
    )
    return res


def kernel(**inputs):
    res = run(inputs, trace=False)
    return assemble(res.results)


# revision 24
# speedup vs baseline: 1.1183x; 1.0110x over previous
"""Distributed attention kernel for 8 TRN2 NeuronCores.

Problem: L=2048, B=2, E=256, H=8 heads, D=32 head-dim, fp32.

Sharding: DP2 over batch x sequence-parallel-4 over query positions.
Core c handles batch c//4, query rows [512*(c%4), 512*(c%4+1)), ALL 8
heads. k/v projections are redundantly computed per batch group (cheap)
and NO collective is needed: each core owns a disjoint output block.

Per-core pipeline (v4 -- cell units, ping-pong score tiles, dual exp):
  - The score work is 128 cells (pass, tk-chunk g, head h) of
    [K=32 d, M=128 tk, N=512 tq]. Cells run THREE at a time as one
    "unit": 3 concurrent PE matmuls on distinct 32-row bands
    (tile_position row tiling), each filling its own PSUM bank (a bank
    shared by concurrently-executing row-tiled matmuls hangs the
    device -- HW-verified).
  - TWO 3-bank score tiles ping-pong between units. Separate pool
    tiles are required: the Tile dep tracker is coarse-grained, so a
    shared tile serializes unit i+1's scores behind unit i's exp read
    (measured +0.7us/unit).
  - softmax exp runs on TWO engines in parallel: ScalarE exact exp via
    the ACT LUT; VectorE a Schraudolph exp2 (one fused mult+add
    tensor_scalar emitting the bf16 BIT PATTERN as int16, ~1.8% rms
    error, softmax-normalized). A minority of units take the DVE path
    so the output error stays ~1.3% (budget 2e-2).
  - PV is software-pipelined one unit behind (the PE is in-order; a PV
    waiting on exp would head-of-line-block the next scores), and
    deferred entirely while the q/k/v projections own the last two
    PSUM banks -- pool lifetimes let the projection psum, the PV
    accumulators, and the final projection accumulators share banks
    6-7 in sequence.
  - PV uses P.T chunks as STATIONARY and [v|1] as moving so O lands in
    natural [tq, d] orientation with the softmax denominator Z as a
    free per-partition column; xbar DMA transposes produce O.T, and
    the Wp projection runs at the tail with the per-head 1/Z folded in
    beforehand (reciprocal + broadcast multiply on DVE).
"""

import math
import os
import sys

import numpy as np

for _p in ("/opt/trn_rl_repo",):
    if _p not in sys.path and os.path.isdir(_p):
        sys.path.insert(0, _p)

import ml_dtypes

import concourse.bass as bass
import concourse.bacc as bacc
import concourse.mybir as mybir
import concourse.tile as tile
from concourse.bass_utils import run_bass_kernel_spmd

dt = mybir.dt
F32 = dt.float32
BF16 = dt.bfloat16
I16 = dt.int16
AF = mybir.ActivationFunctionType
ALU = mybir.AluOpType
BF = ml_dtypes.bfloat16

L, B, E, H, D = 2048, 2, 256, 8, 32
SCALE = float(D) ** -0.5
NCORES = 8
SP = 4            # sequence-parallel ways
TQ = L // SP      # 512 query rows per core
NTK = L // 128    # 16 tk chunks
VW = H * (D + 1)  # v_buf cols per tk chunk: 8x [v_h | 1] = 264
NPASS = 2         # head passes (4 heads each)
CPU = 3           # cells per unit

# Schraudolph exp2: bf16 bits of exp(s*SCALE) ~= int16(s*C1 + C2).
C1 = 128.0 * SCALE * math.log2(math.e)
C2 = 128.0 * (127.0 - 0.0434) + 0.5

_GRAPH = None


def _build_graph():
    nc = bacc.Bacc(
        "TRN2",
        target_bir_lowering=False,
        debug=False,
        enable_asserts=False,
        num_devices=NCORES,
    )

    xqt = nc.declare_dram_parameter("xqt", [E, TQ], BF16, isOutput=False).ap()
    xkt = nc.declare_dram_parameter("xkt", [E, L], BF16, isOutput=False).ap()
    xvt = nc.declare_dram_parameter("xvt", [E, L], BF16, isOutput=False).ap()
    wq = nc.declare_dram_parameter("wq", [E, E], BF16, isOutput=False).ap()
    wk = nc.declare_dram_parameter("wk", [E, E], BF16, isOutput=False).ap()
    wv = nc.declare_dram_parameter("wv", [E, E], BF16, isOutput=False).ap()
    wp = nc.declare_dram_parameter("wp", [E, E], BF16, isOutput=False).ap()
    bq = nc.declare_dram_parameter("bq", [1, E], F32, isOutput=False).ap()
    bk = nc.declare_dram_parameter("bk", [1, E], F32, isOutput=False).ap()
    bv = nc.declare_dram_parameter("bv", [1, E], F32, isOutput=False).ap()
    bp = nc.declare_dram_parameter("bp", [1, E], F32, isOutput=False).ap()
    out = nc.declare_dram_parameter("out", [TQ, E], F32, isOutput=True).ap()

    with tile.TileContext(nc) as tc:
        with (
            tc.tile_pool(name="persist", bufs=1) as pp,
            tc.tile_pool(name="pt", bufs=26) as ptp,
            tc.tile_pool(name="osb", bufs=2) as osbp,
            tc.tile_pool(name="onat", bufs=2) as onatp,
            tc.tile_pool(name="rz", bufs=2) as rzp,
            tc.tile_pool(name="vstage", bufs=4) as vsp,
            tc.tile_pool(name="outsb", bufs=2) as outp,
            tc.tile_pool(name="st", bufs=2, space="PSUM") as stp,
        ):
            # ---------- phase 0: loads ----------
            # weights: tile [128, 2E]; slice e covers W rows [128e, 128e+128)
            w_sb = {}

            def load_w(name, wsrc, eng):
                t = pp.tile([128, 2 * E], BF16, name=f"w{name}", tag=f"w{name}")
                eng.dma_start(
                    out=t[:].rearrange("p (e n) -> p e n", e=2),
                    in_=wsrc.rearrange("(e p) n -> p e n", p=128),
                )
                w_sb[name] = t

            # scalar queue: q-projection inputs first; sync queue: wk +
            # half of xk. v inputs and remaining weights come later.
            load_w("q", wq, nc.scalar)
            xq_sb = []
            for e in range(2):
                t = pp.tile([128, TQ], BF16, name=f"xqt{e}", tag=f"xqt{e}")
                eng = nc.scalar if e == 0 else nc.sync
                eng.dma_start(out=t[:], in_=xqt[e * 128:(e + 1) * 128, :])
                xq_sb.append(t)
            load_w("k", wk, nc.sync)
            xk_sb = [
                pp.tile([128, L], BF16, name=f"xkt{e}", tag=f"xkt{e}")
                for e in range(2)
            ]
            for n in range(4):
                for e in range(2):
                    eng = nc.scalar if e == 0 else nc.sync
                    eng.dma_start(
                        out=xk_sb[e][:, n * 512:(n + 1) * 512],
                        in_=xkt[e * 128:(e + 1) * 128, n * 512:(n + 1) * 512],
                    )
            bq_sb = pp.tile([128, 2], F32)
            nc.gpsimd.dma_start(
                out=bq_sb[:], in_=bq.rearrange("a (c p) -> p (a c)", p=128)
            )
            bv_col = pp.tile([128, 2], BF16)
            nc.gpsimd.dma_start(
                out=bv_col[:], in_=bv.rearrange("a (c p) -> p (a c)", p=128)
            )
            bp_sb = pp.tile([128, E], F32)
            nc.gpsimd.dma_start(out=bp_sb[:], in_=bp.to_broadcast((128, E)))
            load_w("v", wv, nc.gpsimd)
            load_w("p", wp, nc.gpsimd)
            # xv rides the gpsimd SWDGE queue, keeping the two HWDGE
            # queues free for the latency-critical q/k inputs
            xv_sb = [
                pp.tile([128, L], BF16, name=f"xvt{e}", tag=f"xvt{e}")
                for e in range(2)
            ]
            for n in range(4):
                for e in range(2):
                    nc.gpsimd.dma_start(
                        out=xv_sb[e][:, n * 512:(n + 1) * 512],
                        in_=xvt[e * 128:(e + 1) * 128, n * 512:(n + 1) * 512],
                    )

            # warm the exp ACT table AFTER the load triggers are on the
            # scalar queue (the ~2.7us table load must not delay them)
            warm = pp.tile([1, 16], F32)
            nc.vector.memset(warm[:], 0.0)
            nc.scalar.activation(warm[:], warm[:], AF.Exp)

            # ---------- persistent SBUF state ----------
            # kT[hc]: [128 = 4 heads x 32 d (bands 0/32/64/96), 2048 tk]
            kT = [pp.tile([128, L], BF16, name=f"kT{hc}", tag=f"kT{hc}")
                  for hc in range(2)]
            qT = [pp.tile([128, TQ], BF16, name=f"qT{hc}", tag=f"qT{hc}")
                  for hc in range(2)]
            v_buf = pp.tile([128, NTK * VW], BF16)
            nc.gpsimd.memset(v_buf[:], 1.0)

            # ping-pong score tiles: 3 banks each (bank r <-> the r-th
            # row band used by the unit)
            st_ab = [
                stp.tile([128, CPU * 512], F32, name=f"st{i}", tag="st")
                for i in range(2)
            ]

            # ---------- cell/unit machinery ----------
            # cell = (pass, g, h): scores for head 4p+h over tk chunk g,
            # all 512 tq. Units take 3 consecutive cells (distinct h mod
            # 4 -> distinct PE row bands).
            cells = [(p, g, h) for p in range(NPASS) for g in range(NTK)
                     for h in range(4)]
            cursor = [0]        # next cell index
            unit_no = [0]
            pv_pending = []     # descs awaiting PV emission
            pv_enabled = [False]
            po_tiles = {}

            def emit_pv_cell(desc):
                p, g, h, pt, r = desc
                poA, poB = po_tiles[p]
                po = poA if h < 2 else poB
                uu = h % 2
                hh = 4 * p + h
                for m in range(4):
                    nc.tensor.matmul(
                        po[:, uu * 132 + m * 33: uu * 132 + m * 33 + 33],
                        pt[:, r * 512 + m * 128: r * 512 + (m + 1) * 128],
                        v_buf[:, g * VW + hh * (D + 1): g * VW + (hh + 1) * (D + 1)],
                        start=(g == 0 and uu == 0 and m == 0),
                        stop=(g == NTK - 1 and uu == 1 and m == 3),
                        skip_group_check=True,
                    )

            def flush_pv(keep=0):
                while len(pv_pending) > keep:
                    emit_pv_cell(pv_pending.pop(0))

            def emit_unit():
                """scores + exp for the next <=3 cells; queues their PV."""
                lo = cursor[0]
                hi = min(lo + CPU, len(cells))
                if lo >= hi:
                    return False
                cursor[0] = hi
                q = unit_no[0]
                unit_no[0] += 1
                st = st_ab[q % 2]
                ncell = hi - lo
                for r in range(ncell):
                    p, g, h = cells[lo + r]
                    nc.tensor.matmul(
                        st[:, r * 512:(r + 1) * 512],
                        kT[p][32 * h:32 * h + D, g * 128:(g + 1) * 128],
                        qT[p][32 * h:32 * h + D, :],
                        start=True,
                        stop=True,
                        tile_position=(32 * h, 0),
                    )
                pt = ptp.tile([128, CPU * 512], BF16, tag="pt")
                # exp split WITHIN the unit: ScalarE takes the first two
                # cells (exact exp), DVE the third (Schraudolph). Both
                # run concurrently, so the unit's exp latency is the
                # ScalarE instruction (~1.1us), which fits under the
                # two-unit PE budget of the st-tile ping-pong chain.
                ws = min(2, ncell) * 512
                nc.scalar.activation(
                    pt[:, 0:ws], st[:, 0:ws], AF.Exp, scale=SCALE
                )
                if ncell == CPU:
                    nc.vector.tensor_scalar(
                        pt[:, ws:ws + 512].bitcast(I16), st[:, ws:ws + 512],
                        C1, C2, ALU.mult, ALU.add,
                    )
                for r in range(ncell):
                    p, g, h = cells[lo + r]
                    pv_pending.append((p, g, h, pt, r))
                if pv_enabled[0]:
                    flush_pv(keep=2 * CPU)
                return True

            # ---------- projections (psum banks 6-7), interleaved with
            # the first attention units' scores+exp (PV deferred) ----------
            with tc.tile_pool(name="ps", bufs=2, space="PSUM") as psq:
                for hc in range(2):
                    ps = psq.tile([128, TQ], F32, tag="ps")
                    for e in range(2):
                        nc.tensor.matmul(
                            ps[:],
                            w_sb["q"][:, e * E + hc * 128: e * E + (hc + 1) * 128],
                            xq_sb[e][:, :],
                            start=(e == 0),
                            stop=(e == 1),
                        )
                    nc.vector.tensor_scalar_add(
                        qT[hc][:, :], ps[:], bq_sb[:, hc:hc + 1]
                    )
                for n in range(4):
                    for hc in range(2):
                        ps = psq.tile([128, 512], F32, tag="ps")
                        for e in range(2):
                            nc.tensor.matmul(
                                ps[:],
                                w_sb["k"][:, e * E + hc * 128: e * E + (hc + 1) * 128],
                                xk_sb[e][:, n * 512:(n + 1) * 512],
                                start=(e == 0),
                                stop=(e == 1),
                            )
                        # bk dropped: softmax(S + const-per-row) is
                        # invariant, and (q+bq).bk is constant across
                        # keys -> pure copy.
                        nc.vector.tensor_copy(
                            kT[hc][:, n * 512:(n + 1) * 512], ps[:]
                        )
                    for t in range(4 * n, 4 * n + 4):
                        ps = psq.tile([128, E], F32, tag="ps")
                        for e in range(2):
                            nc.tensor.matmul(
                                ps[:],
                                xv_sb[e][:, t * 128:(t + 1) * 128],
                                w_sb["v"][:, e * E:(e + 1) * E],
                                start=(e == 0),
                                stop=(e == 1),
                            )
                        vs = vsp.tile([128, E], BF16, tag="vstage")
                        # bv folds into the output bias (sum of softmax
                        # weights is 1): out += bv @ Wp, added at the
                        # tail -> pure copy.
                        nc.vector.tensor_copy(vs[:], ps[:])
                        nc.sync.dma_start(
                            out=v_buf[:, t * VW:(t + 1) * VW].rearrange(
                                "p (h w) -> p h w", h=H
                            )[:, :, 0:D],
                            in_=vs[:].rearrange("p (h d) -> p h d", h=H),
                        )
                    # attention units whose kT chunks are now projected:
                    # pass-0 cells with g <= 4n+3
                    while cursor[0] <= (4 * n + 4) * 4 - CPU:
                        emit_unit()

            # ---------- PV accumulators take over banks 6-7 ----------
            onat_t = {}
            osb_t = {}

            def finalize(p):
                """normalize + transpose O for pass p (proj at tail)."""
                poA, poB = po_tiles[p]
                onat = onatp.tile([128, TQ], BF16, name=f"onat{p}", tag="onat")
                osb = osbp.tile([128, TQ], BF16, name=f"osb{p}", tag="osb")
                rz = rzp.tile([128, 16], F32, name=f"rz{p}", tag="rz")
                onat_t[p], osb_t[p] = onat, osb
                for idx, po in ((0, poA), (1, poB)):
                    zv = po[:].rearrange("p (b m w) -> p b m w", b=2, m=4)[
                        :, :, :, D:D + 1
                    ]
                    rzo = rz[:, idx * 8:(idx + 1) * 8].rearrange(
                        "p (b m) -> p b m", b=2
                    ).unsqueeze(3)
                    nc.vector.reciprocal(rzo, zv)
                for m in range(4):
                    for idx, po in ((0, poA), (1, poB)):
                        # both uu of this po, m-th chunk: [128, 2, 32]
                        pin = po[:].rearrange(
                            "p (b mm w) -> p b mm w", b=2, w=33
                        )[:, :, m:m + 1, 0:D]
                        rzb = rz[:, idx * 8:(idx + 1) * 8].rearrange(
                            "p (b mm) -> p b mm", b=2
                        )[:, :, m:m + 1].unsqueeze(3).to_broadcast(
                            (128, 2, 1, D)
                        )
                        pout = onat[:].rearrange(
                            "p (mm b w) -> p mm b w", mm=4, b=4
                        )[:, m:m + 1, 2 * idx:2 * idx + 2, :]
                        nc.vector.tensor_tensor(pout, pin, rzb, ALU.mult)
                    eng = nc.sync if m % 2 == 0 else nc.scalar
                    eng.dma_start_transpose(
                        osb[:, m * 128:(m + 1) * 128],
                        onat[:, m * 128:(m + 1) * 128],
                    )

            with tc.tile_pool(name="po", bufs=2, space="PSUM") as pop:
                po_tiles[0] = (
                    pop.tile([128, 264], F32, name="poA0", tag="po"),
                    pop.tile([128, 264], F32, name="poB0", tag="po"),
                )
                pv_enabled[0] = True
                flush_pv(keep=CPU)
                # emit remaining pass-0 cells (units may straddle into
                # pass 1; their pass-1 PVs wait in pv_pending)
                npass0_cells = NTK * 4
                while cursor[0] < npass0_cells:
                    emit_unit()
                while any(d[0] == 0 for d in pv_pending):
                    emit_pv_cell(pv_pending.pop(0))
                finalize(0)
                po_tiles[1] = (
                    pop.tile([128, 264], F32, name="poA1", tag="po"),
                    pop.tile([128, 264], F32, name="poB1", tag="po"),
                )
                flush_pv(keep=CPU)
                while emit_unit():
                    pass
                flush_pv()
                finalize(1)

                # ---------- tail: Wp projection + bias + out DMA ----------
                pjt = [
                    pop.tile([128, 2 * E], F32, name=f"pjt{i}", tag="po")
                    for i in range(2)
                ]
                # from the st pool: its slots are dead at the tail (the
                # po pool's 2 slots still hold the live pjt tiles)
                bbp = stp.tile([128, E], F32, name="bbp", tag="st")
                for e in range(2):
                    nc.tensor.matmul(
                        bbp[:],
                        bv_col[:, e:e + 1].to_broadcast((128, 128)),
                        w_sb["p"][:, e * E:(e + 1) * E],
                        start=(e == 0),
                        stop=(e == 1),
                    )
                bb_sb = pp.tile([128, E], F32, name="bb_sb")
                nc.vector.tensor_tensor(bb_sb[:], bbp[:], bp_sb[:], ALU.add)
                for m in range(4):
                    for p in range(NPASS):
                        nc.tensor.matmul(
                            pjt[m // 2][:, (m % 2) * E:(m % 2 + 1) * E],
                            osb_t[p][:, m * 128:(m + 1) * 128],
                            w_sb["p"][:, p * E:(p + 1) * E],
                            start=(p == 0 and m % 2 == 0),
                            stop=(p == NPASS - 1 and m % 2 == 1),
                            skip_group_check=True,
                        )
                    if m % 2 == 1:
                        # drain this pjt pair as soon as it stops
                        for mm in (m - 1, m):
                            ob = outp.tile([128, E], F32, tag="outsb")
                            nc.vector.tensor_tensor(
                                ob[:],
                                pjt[mm // 2][:, (mm % 2) * E:(mm % 2 + 1) * E],
                                bb_sb[:], ALU.add,
                            )
                            eng = nc.sync if mm % 2 == 0 else nc.scalar
                            eng.dma_start(
                                out=out[mm * 128:(mm + 1) * 128, :], in_=ob[:]
                            )

    return nc


def get_graph():
    global _GRAPH
    if _GRAPH is None:
        nc = _build_graph()
        nc.compile()
        _GRAPH = nc
    return _GRAPH


def make_in_maps(query, key_, value, Wq, bq, Wk, bk, Wv, bv, Wp, bp):
    query = np.asarray(query, np.float32)
    key_ = np.asarray(key_, np.float32)
    value = np.asarray(value, np.float32)
    Wq, Wk, Wv, Wp = (np.asarray(w, np.float32) for w in (Wq, Wk, Wv, Wp))
    bq, bk, bv, bp = (np.asarray(b_, np.float32) for b_ in (bq, bk, bv, bp))

    wq_b = np.ascontiguousarray(Wq).astype(BF)
    wk_b = np.ascontiguousarray(Wk).astype(BF)
    wv_b = np.ascontiguousarray(Wv).astype(BF)
    wp_b = np.ascontiguousarray(Wp).astype(BF)
    xt = {}
    for b in range(B):
        xt[("q", b)] = np.ascontiguousarray(query[:, b, :].T).astype(BF)
        xt[("k", b)] = np.ascontiguousarray(key_[:, b, :].T).astype(BF)
        xt[("v", b)] = np.ascontiguousarray(value[:, b, :].T).astype(BF)

    in_maps = []
    for c in range(NCORES):
        b = c // SP
        p = c % SP
        m = {
            "xqt": np.ascontiguousarray(xt[("q", b)][:, p * TQ:(p + 1) * TQ]),
            "xkt": xt[("k", b)],
            "xvt": xt[("v", b)],
            "wq": wq_b,
            "wk": wk_b,
            "wv": wv_b,
            "wp": wp_b,
            "bq": bq.reshape(1, E).copy(),
            "bk": bk.reshape(1, E).copy(),
            "bv": bv.reshape(1, E).copy(),
            "bp": bp.reshape(1, E).copy(),
        }
        in_maps.append(m)
    return in_maps


def assemble(results):
    out_full = np.empty((L, B, E), np.float32)
    for c in range(NCORES):
        b = c // SP
        p = c % SP
        out_full[p * TQ:(p + 1) * TQ, b, :] = results[c]["out"]
    return out_full


def run(inputs, trace=False, **kw):
    nc = get_graph()
    in_maps = make_in_maps(**inputs)
    res = run_bass_kernel_spmd(
        nc, in_maps, core_ids=list(range(NCORES)), trace=trace, **kw
    )
    return res


def kernel(**inputs):
    res = run(inputs, trace=False)
    return assemble(res.results)


# revision 25
# speedup vs baseline: 1.1204x; 1.0019x over previous
"""Distributed attention kernel for 8 TRN2 NeuronCores.

Problem: L=2048, B=2, E=256, H=8 heads, D=32 head-dim, fp32.

Sharding: DP2 over batch x sequence-parallel-4 over query positions.
Core c handles batch c//4, query rows [512*(c%4), 512*(c%4+1)), ALL 8
heads. k/v projections are redundantly computed per batch group (cheap)
and NO collective is needed: each core owns a disjoint output block.

Per-core pipeline (v4 -- cell units, ping-pong score tiles, dual exp):
  - The score work is 128 cells (pass, tk-chunk g, head h) of
    [K=32 d, M=128 tk, N=512 tq]. Cells run THREE at a time as one
    "unit": 3 concurrent PE matmuls on distinct 32-row bands
    (tile_position row tiling), each filling its own PSUM bank (a bank
    shared by concurrently-executing row-tiled matmuls hangs the
    device -- HW-verified).
  - TWO 3-bank score tiles ping-pong between units. Separate pool
    tiles are required: the Tile dep tracker is coarse-grained, so a
    shared tile serializes unit i+1's scores behind unit i's exp read
    (measured +0.7us/unit).
  - softmax exp runs on TWO engines in parallel: ScalarE exact exp via
    the ACT LUT; VectorE a Schraudolph exp2 (one fused mult+add
    tensor_scalar emitting the bf16 BIT PATTERN as int16, ~1.8% rms
    error, softmax-normalized). A minority of units take the DVE path
    so the output error stays ~1.3% (budget 2e-2).
  - PV is software-pipelined one unit behind (the PE is in-order; a PV
    waiting on exp would head-of-line-block the next scores), and
    deferred entirely while the q/k/v projections own the last two
    PSUM banks -- pool lifetimes let the projection psum, the PV
    accumulators, and the final projection accumulators share banks
    6-7 in sequence.
  - PV uses P.T chunks as STATIONARY and [v|1] as moving so O lands in
    natural [tq, d] orientation with the softmax denominator Z as a
    free per-partition column; xbar DMA transposes produce O.T, and
    the Wp projection runs at the tail with the per-head 1/Z folded in
    beforehand (reciprocal + broadcast multiply on DVE).
"""

import math
import os
import sys

import numpy as np

for _p in ("/opt/trn_rl_repo",):
    if _p not in sys.path and os.path.isdir(_p):
        sys.path.insert(0, _p)

import ml_dtypes

import concourse.bass as bass
import concourse.bacc as bacc
import concourse.mybir as mybir
import concourse.tile as tile
from concourse.bass_utils import run_bass_kernel_spmd

dt = mybir.dt
F32 = dt.float32
BF16 = dt.bfloat16
I16 = dt.int16
AF = mybir.ActivationFunctionType
ALU = mybir.AluOpType
BF = ml_dtypes.bfloat16

L, B, E, H, D = 2048, 2, 256, 8, 32
SCALE = float(D) ** -0.5
NCORES = 8
SP = 4            # sequence-parallel ways
TQ = L // SP      # 512 query rows per core
NTK = L // 128    # 16 tk chunks
VW = H * (D + 1)  # v_buf cols per tk chunk: 8x [v_h | 1] = 264
NPASS = 2         # head passes (4 heads each)
CPU = 3           # cells per unit

# Schraudolph exp2: bf16 bits of exp(s*SCALE) ~= int16(s*C1 + C2).
C1 = 128.0 * SCALE * math.log2(math.e)
C2 = 128.0 * (127.0 - 0.0434) + 0.5

_GRAPH = None


def _build_graph():
    nc = bacc.Bacc(
        "TRN2",
        target_bir_lowering=False,
        debug=False,
        enable_asserts=False,
        num_devices=NCORES,
    )

    xqt = nc.declare_dram_parameter("xqt", [E, TQ], BF16, isOutput=False).ap()
    xkt = nc.declare_dram_parameter("xkt", [E, L], BF16, isOutput=False).ap()
    xvt = nc.declare_dram_parameter("xvt", [E, L], BF16, isOutput=False).ap()
    wq = nc.declare_dram_parameter("wq", [E, E], BF16, isOutput=False).ap()
    wk = nc.declare_dram_parameter("wk", [E, E], BF16, isOutput=False).ap()
    wv = nc.declare_dram_parameter("wv", [E, E], BF16, isOutput=False).ap()
    wp = nc.declare_dram_parameter("wp", [E, E], BF16, isOutput=False).ap()
    bq = nc.declare_dram_parameter("bq", [1, E], F32, isOutput=False).ap()
    bk = nc.declare_dram_parameter("bk", [1, E], F32, isOutput=False).ap()
    bv = nc.declare_dram_parameter("bv", [1, E], F32, isOutput=False).ap()
    bp = nc.declare_dram_parameter("bp", [1, E], F32, isOutput=False).ap()
    out = nc.declare_dram_parameter("out", [TQ, E], F32, isOutput=True).ap()

    with tile.TileContext(nc) as tc:
        with (
            tc.tile_pool(name="persist", bufs=1) as pp,
            tc.tile_pool(name="pt", bufs=26) as ptp,
            tc.tile_pool(name="osb", bufs=2) as osbp,
            tc.tile_pool(name="onat", bufs=2) as onatp,
            tc.tile_pool(name="rz", bufs=2) as rzp,
            tc.tile_pool(name="vstage", bufs=4) as vsp,
            tc.tile_pool(name="outsb", bufs=2) as outp,
            tc.tile_pool(name="st", bufs=2, space="PSUM") as stp,
        ):
            # ---------- phase 0: loads ----------
            # weights: tile [128, 2E]; slice e covers W rows [128e, 128e+128)
            w_sb = {}

            def load_w(name, wsrc, eng, split=False):
                t = pp.tile([128, 2 * E], BF16, name=f"w{name}", tag=f"w{name}")
                if split:
                    # two plain contiguous DMAs (faster than one strided
                    # gather; the first e-chunk unblocks its matmul early)
                    for e in range(2):
                        eng.dma_start(
                            out=t[:, e * E:(e + 1) * E],
                            in_=wsrc[e * 128:(e + 1) * 128, :],
                        )
                else:
                    eng.dma_start(
                        out=t[:].rearrange("p (e n) -> p e n", e=2),
                        in_=wsrc.rearrange("(e p) n -> p e n", p=128),
                    )
                w_sb[name] = t

            # scalar queue: q-projection inputs first; sync queue: wk +
            # half of xk. v inputs and remaining weights come later.
            load_w("q", wq, nc.scalar, split=True)
            xq_sb = []
            for e in range(2):
                t = pp.tile([128, TQ], BF16, name=f"xqt{e}", tag=f"xqt{e}")
                eng = nc.scalar if e == 0 else nc.sync
                eng.dma_start(out=t[:], in_=xqt[e * 128:(e + 1) * 128, :])
                xq_sb.append(t)
            load_w("k", wk, nc.sync, split=True)
            xk_sb = [
                pp.tile([128, L], BF16, name=f"xkt{e}", tag=f"xkt{e}")
                for e in range(2)
            ]
            for n in range(4):
                for e in range(2):
                    eng = nc.scalar if e == 0 else nc.sync
                    eng.dma_start(
                        out=xk_sb[e][:, n * 512:(n + 1) * 512],
                        in_=xkt[e * 128:(e + 1) * 128, n * 512:(n + 1) * 512],
                    )
            bq_sb = pp.tile([128, 2], F32)
            nc.gpsimd.dma_start(
                out=bq_sb[:], in_=bq.rearrange("a (c p) -> p (a c)", p=128)
            )
            bv_col = pp.tile([128, 2], BF16)
            nc.gpsimd.dma_start(
                out=bv_col[:], in_=bv.rearrange("a (c p) -> p (a c)", p=128)
            )
            bp_sb = pp.tile([128, E], F32)
            nc.gpsimd.dma_start(out=bp_sb[:], in_=bp.to_broadcast((128, E)))
            load_w("v", wv, nc.gpsimd)
            load_w("p", wp, nc.gpsimd)
            # xv rides the gpsimd SWDGE queue, keeping the two HWDGE
            # queues free for the latency-critical q/k inputs
            xv_sb = [
                pp.tile([128, L], BF16, name=f"xvt{e}", tag=f"xvt{e}")
                for e in range(2)
            ]
            for n in range(4):
                for e in range(2):
                    nc.gpsimd.dma_start(
                        out=xv_sb[e][:, n * 512:(n + 1) * 512],
                        in_=xvt[e * 128:(e + 1) * 128, n * 512:(n + 1) * 512],
                    )

            # warm the exp ACT table AFTER the load triggers are on the
            # scalar queue (the ~2.7us table load must not delay them)
            warm = pp.tile([1, 16], F32)
            nc.vector.memset(warm[:], 0.0)
            nc.scalar.activation(warm[:], warm[:], AF.Exp)

            # ---------- persistent SBUF state ----------
            # kT[hc]: [128 = 4 heads x 32 d (bands 0/32/64/96), 2048 tk]
            kT = [pp.tile([128, L], BF16, name=f"kT{hc}", tag=f"kT{hc}")
                  for hc in range(2)]
            qT = [pp.tile([128, TQ], BF16, name=f"qT{hc}", tag=f"qT{hc}")
                  for hc in range(2)]
            v_buf = pp.tile([128, NTK * VW], BF16)
            nc.gpsimd.memset(v_buf[:], 1.0)

            # ping-pong score tiles: 3 banks each (bank r <-> the r-th
            # row band used by the unit)
            st_ab = [
                stp.tile([128, CPU * 512], F32, name=f"st{i}", tag="st")
                for i in range(2)
            ]

            # ---------- cell/unit machinery ----------
            # cell = (pass, g, h): scores for head 4p+h over tk chunk g,
            # all 512 tq. Units take 3 consecutive cells (distinct h mod
            # 4 -> distinct PE row bands).
            cells = [(p, g, h) for p in range(NPASS) for g in range(NTK)
                     for h in range(4)]
            cursor = [0]        # next cell index
            unit_no = [0]
            pv_pending = []     # descs awaiting PV emission
            pv_enabled = [False]
            po_tiles = {}

            def emit_pv_cell(desc):
                p, g, h, pt, r = desc
                poA, poB = po_tiles[p]
                po = poA if h < 2 else poB
                uu = h % 2
                hh = 4 * p + h
                for m in range(4):
                    nc.tensor.matmul(
                        po[:, uu * 132 + m * 33: uu * 132 + m * 33 + 33],
                        pt[:, r * 512 + m * 128: r * 512 + (m + 1) * 128],
                        v_buf[:, g * VW + hh * (D + 1): g * VW + (hh + 1) * (D + 1)],
                        start=(g == 0 and uu == 0 and m == 0),
                        stop=(g == NTK - 1 and uu == 1 and m == 3),
                        skip_group_check=True,
                    )

            def flush_pv(keep=0):
                while len(pv_pending) > keep:
                    emit_pv_cell(pv_pending.pop(0))

            def emit_unit():
                """scores + exp for the next <=3 cells; queues their PV."""
                lo = cursor[0]
                hi = min(lo + CPU, len(cells))
                if lo >= hi:
                    return False
                cursor[0] = hi
                q = unit_no[0]
                unit_no[0] += 1
                st = st_ab[q % 2]
                ncell = hi - lo
                for r in range(ncell):
                    p, g, h = cells[lo + r]
                    nc.tensor.matmul(
                        st[:, r * 512:(r + 1) * 512],
                        kT[p][32 * h:32 * h + D, g * 128:(g + 1) * 128],
                        qT[p][32 * h:32 * h + D, :],
                        start=True,
                        stop=True,
                        tile_position=(32 * h, 0),
                    )
                pt = ptp.tile([128, CPU * 512], BF16, tag="pt")
                # exp split WITHIN the unit: ScalarE takes the first two
                # cells (exact exp), DVE the third (Schraudolph). Both
                # run concurrently, so the unit's exp latency is the
                # ScalarE instruction (~1.1us), which fits under the
                # two-unit PE budget of the st-tile ping-pong chain.
                ws = min(2, ncell) * 512
                nc.scalar.activation(
                    pt[:, 0:ws], st[:, 0:ws], AF.Exp, scale=SCALE
                )
                if ncell == CPU:
                    nc.vector.tensor_scalar(
                        pt[:, ws:ws + 512].bitcast(I16), st[:, ws:ws + 512],
                        C1, C2, ALU.mult, ALU.add,
                    )
                for r in range(ncell):
                    p, g, h = cells[lo + r]
                    pv_pending.append((p, g, h, pt, r))
                if pv_enabled[0]:
                    flush_pv(keep=2 * CPU)
                return True

            # ---------- projections (psum banks 6-7), interleaved with
            # the first attention units' scores+exp (PV deferred) ----------
            with tc.tile_pool(name="ps", bufs=2, space="PSUM") as psq:
                for hc in range(2):
                    ps = psq.tile([128, TQ], F32, tag="ps")
                    for e in range(2):
                        nc.tensor.matmul(
                            ps[:],
                            w_sb["q"][:, e * E + hc * 128: e * E + (hc + 1) * 128],
                            xq_sb[e][:, :],
                            start=(e == 0),
                            stop=(e == 1),
                        )
                    nc.vector.tensor_scalar_add(
                        qT[hc][:, :], ps[:], bq_sb[:, hc:hc + 1]
                    )
                for n in range(4):
                    for hc in range(2):
                        ps = psq.tile([128, 512], F32, tag="ps")
                        for e in range(2):
                            nc.tensor.matmul(
                                ps[:],
                                w_sb["k"][:, e * E + hc * 128: e * E + (hc + 1) * 128],
                                xk_sb[e][:, n * 512:(n + 1) * 512],
                                start=(e == 0),
                                stop=(e == 1),
                            )
                        # bk dropped: softmax(S + const-per-row) is
                        # invariant, and (q+bq).bk is constant across
                        # keys -> pure copy.
                        nc.vector.tensor_copy(
                            kT[hc][:, n * 512:(n + 1) * 512], ps[:]
                        )
                    for t in range(4 * n, 4 * n + 4):
                        ps = psq.tile([128, E], F32, tag="ps")
                        for e in range(2):
                            nc.tensor.matmul(
                                ps[:],
                                xv_sb[e][:, t * 128:(t + 1) * 128],
                                w_sb["v"][:, e * E:(e + 1) * E],
                                start=(e == 0),
                                stop=(e == 1),
                            )
                        vs = vsp.tile([128, E], BF16, tag="vstage")
                        # bv folds into the output bias (sum of softmax
                        # weights is 1): out += bv @ Wp, added at the
                        # tail -> pure copy.
                        nc.vector.tensor_copy(vs[:], ps[:])
                        nc.sync.dma_start(
                            out=v_buf[:, t * VW:(t + 1) * VW].rearrange(
                                "p (h w) -> p h w", h=H
                            )[:, :, 0:D],
                            in_=vs[:].rearrange("p (h d) -> p h d", h=H),
                        )
                    # attention units whose kT chunks are now projected:
                    # pass-0 cells with g <= 4n+3
                    while cursor[0] <= (4 * n + 4) * 4 - CPU:
                        emit_unit()

            # ---------- PV accumulators take over banks 6-7 ----------
            onat_t = {}
            osb_t = {}

            def finalize(p):
                """normalize + transpose O for pass p (proj at tail)."""
                poA, poB = po_tiles[p]
                onat = onatp.tile([128, TQ], BF16, name=f"onat{p}", tag="onat")
                osb = osbp.tile([128, TQ], BF16, name=f"osb{p}", tag="osb")
                rz = rzp.tile([128, 16], F32, name=f"rz{p}", tag="rz")
                onat_t[p], osb_t[p] = onat, osb
                for idx, po in ((0, poA), (1, poB)):
                    zv = po[:].rearrange("p (b m w) -> p b m w", b=2, m=4)[
                        :, :, :, D:D + 1
                    ]
                    rzo = rz[:, idx * 8:(idx + 1) * 8].rearrange(
                        "p (b m) -> p b m", b=2
                    ).unsqueeze(3)
                    nc.vector.reciprocal(rzo, zv)
                for m in range(4):
                    for idx, po in ((0, poA), (1, poB)):
                        # both uu of this po, m-th chunk: [128, 2, 32]
                        pin = po[:].rearrange(
                            "p (b mm w) -> p b mm w", b=2, w=33
                        )[:, :, m:m + 1, 0:D]
                        rzb = rz[:, idx * 8:(idx + 1) * 8].rearrange(
                            "p (b mm) -> p b mm", b=2
                        )[:, :, m:m + 1].unsqueeze(3).to_broadcast(
                            (128, 2, 1, D)
                        )
                        pout = onat[:].rearrange(
                            "p (mm b w) -> p mm b w", mm=4, b=4
                        )[:, m:m + 1, 2 * idx:2 * idx + 2, :]
                        nc.vector.tensor_tensor(pout, pin, rzb, ALU.mult)
                    eng = nc.sync if m % 2 == 0 else nc.scalar
                    eng.dma_start_transpose(
                        osb[:, m * 128:(m + 1) * 128],
                        onat[:, m * 128:(m + 1) * 128],
                    )

            with tc.tile_pool(name="po", bufs=2, space="PSUM") as pop:
                po_tiles[0] = (
                    pop.tile([128, 264], F32, name="poA0", tag="po"),
                    pop.tile([128, 264], F32, name="poB0", tag="po"),
                )
                pv_enabled[0] = True
                flush_pv(keep=CPU)
                # emit remaining pass-0 cells (units may straddle into
                # pass 1; their pass-1 PVs wait in pv_pending)
                npass0_cells = NTK * 4
                while cursor[0] < npass0_cells:
                    emit_unit()
                while any(d[0] == 0 for d in pv_pending):
                    emit_pv_cell(pv_pending.pop(0))
                finalize(0)
                po_tiles[1] = (
                    pop.tile([128, 264], F32, name="poA1", tag="po"),
                    pop.tile([128, 264], F32, name="poB1", tag="po"),
                )
                flush_pv(keep=CPU)
                while emit_unit():
                    pass
                flush_pv()
                finalize(1)

                # ---------- tail: Wp projection + bias + out DMA ----------
                pjt = [
                    pop.tile([128, 2 * E], F32, name=f"pjt{i}", tag="po")
                    for i in range(2)
                ]
                # from the st pool: its slots are dead at the tail (the
                # po pool's 2 slots still hold the live pjt tiles)
                bbp = stp.tile([128, E], F32, name="bbp", tag="st")
                for e in range(2):
                    nc.tensor.matmul(
                        bbp[:],
                        bv_col[:, e:e + 1].to_broadcast((128, 128)),
                        w_sb["p"][:, e * E:(e + 1) * E],
                        start=(e == 0),
                        stop=(e == 1),
                    )
                bb_sb = pp.tile([128, E], F32, name="bb_sb")
                nc.vector.tensor_tensor(bb_sb[:], bbp[:], bp_sb[:], ALU.add)
                for m in range(4):
                    for p in range(NPASS):
                        nc.tensor.matmul(
                            pjt[m // 2][:, (m % 2) * E:(m % 2 + 1) * E],
                            osb_t[p][:, m * 128:(m + 1) * 128],
                            w_sb["p"][:, p * E:(p + 1) * E],
                            start=(p == 0 and m % 2 == 0),
                            stop=(p == NPASS - 1 and m % 2 == 1),
                            skip_group_check=True,
                        )
                    if m % 2 == 1:
                        # drain this pjt pair as soon as it stops
                        for mm in (m - 1, m):
                            ob = outp.tile([128, E], F32, tag="outsb")
                            nc.vector.tensor_tensor(
                                ob[:],
                                pjt[mm // 2][:, (mm % 2) * E:(mm % 2 + 1) * E],
                                bb_sb[:], ALU.add,
                            )
                            eng = nc.sync if mm % 2 == 0 else nc.scalar
                            eng.dma_start(
                                out=out[mm * 128:(mm + 1) * 128, :], in_=ob[:]
                            )

    return nc


def get_graph():
    global _GRAPH
    if _GRAPH is None:
        nc = _build_graph()
        nc.compile()
        _GRAPH = nc
    return _GRAPH


def make_in_maps(query, key_, value, Wq, bq, Wk, bk, Wv, bv, Wp, bp):
    query = np.asarray(query, np.float32)
    key_ = np.asarray(key_, np.float32)
    value = np.asarray(value, np.float32)
    Wq, Wk, Wv, Wp = (np.asarray(w, np.float32) for w in (Wq, Wk, Wv, Wp))
    bq, bk, bv, bp = (np.asarray(b_, np.float32) for b_ in (bq, bk, bv, bp))

    wq_b = np.ascontiguousarray(Wq).astype(BF)
    wk_b = np.ascontiguousarray(Wk).astype(BF)
    wv_b = np.ascontiguousarray(Wv).astype(BF)
    wp_b = np.ascontiguousarray(Wp).astype(BF)
    xt = {}
    for b in range(B):
        xt[("q", b)] = np.ascontiguousarray(query[:, b, :].T).astype(BF)
        xt[("k", b)] = np.ascontiguousarray(key_[:, b, :].T).astype(BF)
        xt[("v", b)] = np.ascontiguousarray(value[:, b, :].T).astype(BF)

    in_maps = []
    for c in range(NCORES):
        b = c // SP
        p = c % SP
        m = {
            "xqt": np.ascontiguousarray(xt[("q", b)][:, p * TQ:(p + 1) * TQ]),
            "xkt": xt[("k", b)],
            "xvt": xt[("v", b)],
            "wq": wq_b,
            "wk": wk_b,
            "wv": wv_b,
            "wp": wp_b,
            "bq": bq.reshape(1, E).copy(),
            "bk": bk.reshape(1, E).copy(),
            "bv": bv.reshape(1, E).copy(),
            "bp": bp.reshape(1, E).copy(),
        }
        in_maps.append(m)
    return in_maps


def assemble(results):
    out_full = np.empty((L, B, E), np.float32)
    for c in range(NCORES):
        b = c // SP
        p = c % SP
        out_full[p * TQ:(p + 1) * TQ, b, :] = results[c]["out"]
    return out_full


def run(inputs, trace=False, **kw):
    nc = get_graph()
    in_maps = make_in_maps(**inputs)
    res = run_bass_kernel_spmd(
        nc, in_maps, core_ids=list(range(NCORES)), trace=trace, **kw
    )
    return res


def kernel(**inputs):
    res = run(inputs, trace=False)
    return assemble(res.results)


# revision 26
# speedup vs baseline: 1.1394x; 1.0170x over previous
"""Distributed attention kernel for 8 TRN2 NeuronCores.

Problem: L=2048, B=2, E=256, H=8 heads, D=32 head-dim, fp32.

Sharding: DP2 over batch x sequence-parallel-4 over query positions.
Core c handles batch c//4, query rows [512*(c%4), 512*(c%4+1)), ALL 8
heads. k/v projections are redundantly computed per batch group (cheap)
and NO collective is needed: each core owns a disjoint output block.

Per-core pipeline (v4 -- cell units, ping-pong score tiles, dual exp):
  - The score work is 128 cells (pass, tk-chunk g, head h) of
    [K=32 d, M=128 tk, N=512 tq]. Cells run THREE at a time as one
    "unit": 3 concurrent PE matmuls on distinct 32-row bands
    (tile_position row tiling), each filling its own PSUM bank (a bank
    shared by concurrently-executing row-tiled matmuls hangs the
    device -- HW-verified).
  - TWO 3-bank score tiles ping-pong between units. Separate pool
    tiles are required: the Tile dep tracker is coarse-grained, so a
    shared tile serializes unit i+1's scores behind unit i's exp read
    (measured +0.7us/unit).
  - softmax exp runs on TWO engines in parallel: ScalarE exact exp via
    the ACT LUT; VectorE a Schraudolph exp2 (one fused mult+add
    tensor_scalar emitting the bf16 BIT PATTERN as int16, ~1.8% rms
    error, softmax-normalized). A minority of units take the DVE path
    so the output error stays ~1.3% (budget 2e-2).
  - PV is software-pipelined one unit behind (the PE is in-order; a PV
    waiting on exp would head-of-line-block the next scores), and
    deferred entirely while the q/k/v projections own the last two
    PSUM banks -- pool lifetimes let the projection psum, the PV
    accumulators, and the final projection accumulators share banks
    6-7 in sequence.
  - PV uses P.T chunks as STATIONARY and [v|1] as moving so O lands in
    natural [tq, d] orientation with the softmax denominator Z as a
    free per-partition column; xbar DMA transposes produce O.T, and
    the Wp projection runs at the tail with the per-head 1/Z folded in
    beforehand (reciprocal + broadcast multiply on DVE).
"""

import math
import os
import sys

import numpy as np

for _p in ("/opt/trn_rl_repo",):
    if _p not in sys.path and os.path.isdir(_p):
        sys.path.insert(0, _p)

import ml_dtypes

import concourse.bass as bass
import concourse.bacc as bacc
import concourse.mybir as mybir
import concourse.tile as tile
from concourse.bass_utils import run_bass_kernel_spmd

dt = mybir.dt
F32 = dt.float32
BF16 = dt.bfloat16
I16 = dt.int16
AF = mybir.ActivationFunctionType
ALU = mybir.AluOpType
BF = ml_dtypes.bfloat16

L, B, E, H, D = 2048, 2, 256, 8, 32
SCALE = float(D) ** -0.5
NCORES = 8
SP = 4            # sequence-parallel ways
TQ = L // SP      # 512 query rows per core
NTK = L // 128    # 16 tk chunks
VW = H * (D + 1)  # v_buf cols per tk chunk: 8x [v_h | 1] = 264
NPASS = 2         # head passes (4 heads each)
CPU = 3           # cells per unit

# Schraudolph exp2: bf16 bits of exp(s*SCALE) ~= int16(s*C1 + C2).
C1 = 128.0 * SCALE * math.log2(math.e)
C2 = 128.0 * (127.0 - 0.0434) + 0.5

_GRAPH = None


def _build_graph():
    nc = bacc.Bacc(
        "TRN2",
        target_bir_lowering=False,
        debug=False,
        enable_asserts=False,
        num_devices=NCORES,
    )

    xqt = nc.declare_dram_parameter("xqt", [E, TQ], BF16, isOutput=False).ap()
    xkt = nc.declare_dram_parameter("xkt", [E, L], BF16, isOutput=False).ap()
    xvt = nc.declare_dram_parameter("xvt", [E, L], BF16, isOutput=False).ap()
    wq = nc.declare_dram_parameter("wq", [E, E], BF16, isOutput=False).ap()
    wk = nc.declare_dram_parameter("wk", [E, E], BF16, isOutput=False).ap()
    wv = nc.declare_dram_parameter("wv", [E, E], BF16, isOutput=False).ap()
    wp = nc.declare_dram_parameter("wp", [E, E], BF16, isOutput=False).ap()
    bq = nc.declare_dram_parameter("bq", [1, E], F32, isOutput=False).ap()
    bk = nc.declare_dram_parameter("bk", [1, E], F32, isOutput=False).ap()
    bv = nc.declare_dram_parameter("bv", [1, E], F32, isOutput=False).ap()
    bp = nc.declare_dram_parameter("bp", [1, E], F32, isOutput=False).ap()
    out = nc.declare_dram_parameter("out", [TQ, E], F32, isOutput=True).ap()

    with tile.TileContext(nc) as tc:
        with (
            tc.tile_pool(name="persist", bufs=1) as pp,
            tc.tile_pool(name="pt", bufs=26) as ptp,
            tc.tile_pool(name="osb", bufs=2) as osbp,
            tc.tile_pool(name="onat", bufs=2) as onatp,
            tc.tile_pool(name="rz", bufs=2) as rzp,
            tc.tile_pool(name="vstage", bufs=4) as vsp,
            tc.tile_pool(name="outsb", bufs=2) as outp,
            tc.tile_pool(name="st", bufs=2, space="PSUM") as stp,
        ):
            # ---------- phase 0: loads ----------
            # weights: tile [128, 2E]; slice e covers W rows [128e, 128e+128)
            w_sb = {}

            def load_w(name, wsrc, eng, split=False):
                t = pp.tile([128, 2 * E], BF16, name=f"w{name}", tag=f"w{name}")
                if split:
                    # two plain contiguous DMAs (faster than one strided
                    # gather; the first e-chunk unblocks its matmul early)
                    for e in range(2):
                        eng.dma_start(
                            out=t[:, e * E:(e + 1) * E],
                            in_=wsrc[e * 128:(e + 1) * 128, :],
                        )
                else:
                    eng.dma_start(
                        out=t[:].rearrange("p (e n) -> p e n", e=2),
                        in_=wsrc.rearrange("(e p) n -> p e n", p=128),
                    )
                w_sb[name] = t

            # scalar queue: q-projection inputs first; sync queue: wk +
            # half of xk. v inputs and remaining weights come later.
            load_w("q", wq, nc.scalar, split=True)
            xq_sb = []
            for e in range(2):
                t = pp.tile([128, TQ], BF16, name=f"xqt{e}", tag=f"xqt{e}")
                eng = nc.scalar if e == 0 else nc.sync
                eng.dma_start(out=t[:], in_=xqt[e * 128:(e + 1) * 128, :])
                xq_sb.append(t)
            load_w("k", wk, nc.sync, split=True)
            xk_sb = [
                pp.tile([128, L], BF16, name=f"xkt{e}", tag=f"xkt{e}")
                for e in range(2)
            ]
            for n in range(4):
                for e in range(2):
                    eng = nc.scalar if e == 0 else nc.sync
                    eng.dma_start(
                        out=xk_sb[e][:, n * 512:(n + 1) * 512],
                        in_=xkt[e * 128:(e + 1) * 128, n * 512:(n + 1) * 512],
                    )
            bq_sb = pp.tile([128, 2], F32)
            nc.gpsimd.dma_start(
                out=bq_sb[:], in_=bq.rearrange("a (c p) -> p (a c)", p=128)
            )
            bv_col = pp.tile([128, 2], BF16)
            nc.gpsimd.dma_start(
                out=bv_col[:], in_=bv.rearrange("a (c p) -> p (a c)", p=128)
            )
            bp_sb = pp.tile([128, E], F32)
            nc.gpsimd.dma_start(out=bp_sb[:], in_=bp.to_broadcast((128, E)))
            load_w("v", wv, nc.gpsimd)
            load_w("p", wp, nc.gpsimd)
            # xv rides the gpsimd SWDGE queue, keeping the two HWDGE
            # queues free for the latency-critical q/k inputs
            xv_sb = [
                pp.tile([128, L], BF16, name=f"xvt{e}", tag=f"xvt{e}")
                for e in range(2)
            ]
            for n in range(4):
                for e in range(2):
                    nc.gpsimd.dma_start(
                        out=xv_sb[e][:, n * 512:(n + 1) * 512],
                        in_=xvt[e * 128:(e + 1) * 128, n * 512:(n + 1) * 512],
                    )

            # warm the exp ACT table AFTER the load triggers are on the
            # scalar queue (the ~2.7us table load must not delay them)
            warm = pp.tile([1, 16], F32)
            nc.vector.memset(warm[:], 0.0)
            nc.scalar.activation(warm[:], warm[:], AF.Exp)

            # ---------- persistent SBUF state ----------
            # kT[hc]: [128 = 4 heads x 32 d (bands 0/32/64/96), 2048 tk]
            kT = [pp.tile([128, L], BF16, name=f"kT{hc}", tag=f"kT{hc}")
                  for hc in range(2)]
            qT = [pp.tile([128, TQ], BF16, name=f"qT{hc}", tag=f"qT{hc}")
                  for hc in range(2)]
            v_buf = pp.tile([128, NTK * VW], BF16)
            nc.gpsimd.memset(v_buf[:], 1.0)

            # ping-pong score tiles: 3 banks each (bank r <-> the r-th
            # row band used by the unit)
            st_ab = [
                stp.tile([128, CPU * 512], F32, name=f"st{i}", tag="st")
                for i in range(2)
            ]

            # ---------- cell/unit machinery ----------
            # cell = (pass, g, h): scores for head 4p+h over tk chunk g,
            # all 512 tq. Units take 3 consecutive cells (distinct h mod
            # 4 -> distinct PE row bands).
            cells = [(p, g, h) for p in range(NPASS) for g in range(NTK)
                     for h in range(4)]
            cursor = [0]        # next cell index
            unit_no = [0]
            pv_pending = []     # descs awaiting PV emission
            pv_enabled = [False]
            po_tiles = {}

            def emit_pv_cell(desc):
                p, g, h, pt, r = desc
                poA, poB = po_tiles[p]
                po = poA if h < 2 else poB
                uu = h % 2
                hh = 4 * p + h
                for m in range(4):
                    nc.tensor.matmul(
                        po[:, uu * 132 + m * 33: uu * 132 + m * 33 + 33],
                        pt[:, r * 512 + m * 128: r * 512 + (m + 1) * 128],
                        v_buf[:, g * VW + hh * (D + 1): g * VW + (hh + 1) * (D + 1)],
                        start=(g == 0 and uu == 0 and m == 0),
                        stop=(g == NTK - 1 and uu == 1 and m == 3),
                        skip_group_check=True,
                    )

            def flush_pv(keep=0):
                while len(pv_pending) > keep:
                    emit_pv_cell(pv_pending.pop(0))

            def emit_unit():
                """scores + exp for the next <=3 cells; queues their PV."""
                lo = cursor[0]
                hi = min(lo + CPU, len(cells))
                if lo >= hi:
                    return False
                cursor[0] = hi
                q = unit_no[0]
                unit_no[0] += 1
                st = st_ab[q % 2]
                ncell = hi - lo
                for r in range(ncell):
                    p, g, h = cells[lo + r]
                    nc.tensor.matmul(
                        st[:, r * 512:(r + 1) * 512],
                        kT[p][32 * h:32 * h + D, g * 128:(g + 1) * 128],
                        qT[p][32 * h:32 * h + D, :],
                        start=True,
                        stop=True,
                        tile_position=(32 * h, 0),
                    )
                pt = ptp.tile([128, CPU * 512], BF16, tag="pt")
                # exp split WITHIN the unit: ScalarE takes the first two
                # cells (exact exp), DVE the third (Schraudolph). Both
                # run concurrently, so the unit's exp latency is the
                # ScalarE instruction (~1.1us), which fits under the
                # two-unit PE budget of the st-tile ping-pong chain.
                ws = min(2, ncell) * 512
                nc.scalar.activation(
                    pt[:, 0:ws], st[:, 0:ws], AF.Exp, scale=SCALE
                )
                if ncell == CPU:
                    nc.vector.tensor_scalar(
                        pt[:, ws:ws + 512].bitcast(I16), st[:, ws:ws + 512],
                        C1, C2, ALU.mult, ALU.add,
                    )
                for r in range(ncell):
                    p, g, h = cells[lo + r]
                    pv_pending.append((p, g, h, pt, r))
                if pv_enabled[0]:
                    flush_pv(keep=2 * CPU)
                return True

            # ---------- projections (psum banks 6-7), interleaved with
            # the first attention units' scores+exp (PV deferred) ----------
            with tc.tile_pool(name="ps", bufs=2, space="PSUM") as psq:
                def q_proj():
                    for hc in range(2):
                        ps = psq.tile([128, TQ], F32, tag="ps")
                        for e in range(2):
                            nc.tensor.matmul(
                                ps[:],
                                w_sb["q"][:, e * E + hc * 128: e * E + (hc + 1) * 128],
                                xq_sb[e][:, :],
                                start=(e == 0),
                                stop=(e == 1),
                            )
                        nc.vector.tensor_scalar_add(
                            qT[hc][:, :], ps[:], bq_sb[:, hc:hc + 1]
                        )
                for n in range(4):
                    for hc in range(2):
                        ps = psq.tile([128, 512], F32, tag="ps")
                        for e in range(2):
                            nc.tensor.matmul(
                                ps[:],
                                w_sb["k"][:, e * E + hc * 128: e * E + (hc + 1) * 128],
                                xk_sb[e][:, n * 512:(n + 1) * 512],
                                start=(e == 0),
                                stop=(e == 1),
                            )
                        # bk dropped: softmax(S + const-per-row) is
                        # invariant, and (q+bq).bk is constant across
                        # keys -> pure copy.
                        nc.vector.tensor_copy(
                            kT[hc][:, n * 512:(n + 1) * 512], ps[:]
                        )
                    if n == 0:
                        # q-proj after the first k chunk: the PE has k
                        # work while the q inputs finish landing
                        q_proj()
                    for t in range(4 * n, 4 * n + 4):
                        ps = psq.tile([128, E], F32, tag="ps")
                        for e in range(2):
                            nc.tensor.matmul(
                                ps[:],
                                xv_sb[e][:, t * 128:(t + 1) * 128],
                                w_sb["v"][:, e * E:(e + 1) * E],
                                start=(e == 0),
                                stop=(e == 1),
                            )
                        vs = vsp.tile([128, E], BF16, tag="vstage")
                        # bv folds into the output bias (sum of softmax
                        # weights is 1): out += bv @ Wp, added at the
                        # tail -> pure copy.
                        nc.vector.tensor_copy(vs[:], ps[:])
                        nc.sync.dma_start(
                            out=v_buf[:, t * VW:(t + 1) * VW].rearrange(
                                "p (h w) -> p h w", h=H
                            )[:, :, 0:D],
                            in_=vs[:].rearrange("p (h d) -> p h d", h=H),
                        )
                    # attention units whose kT chunks are now projected:
                    # pass-0 cells with g <= 4n+3
                    while cursor[0] <= (4 * n + 4) * 4 - CPU:
                        emit_unit()

            # ---------- PV accumulators take over banks 6-7 ----------
            onat_t = {}
            osb_t = {}

            def finalize(p):
                """normalize + transpose O for pass p (proj at tail)."""
                poA, poB = po_tiles[p]
                onat = onatp.tile([128, TQ], BF16, name=f"onat{p}", tag="onat")
                osb = osbp.tile([128, TQ], BF16, name=f"osb{p}", tag="osb")
                rz = rzp.tile([128, 16], F32, name=f"rz{p}", tag="rz")
                onat_t[p], osb_t[p] = onat, osb
                for idx, po in ((0, poA), (1, poB)):
                    zv = po[:].rearrange("p (b m w) -> p b m w", b=2, m=4)[
                        :, :, :, D:D + 1
                    ]
                    rzo = rz[:, idx * 8:(idx + 1) * 8].rearrange(
                        "p (b m) -> p b m", b=2
                    ).unsqueeze(3)
                    nc.vector.reciprocal(rzo, zv)
                for m in range(4):
                    for idx, po in ((0, poA), (1, poB)):
                        # both uu of this po, m-th chunk: [128, 2, 32]
                        pin = po[:].rearrange(
                            "p (b mm w) -> p b mm w", b=2, w=33
                        )[:, :, m:m + 1, 0:D]
                        rzb = rz[:, idx * 8:(idx + 1) * 8].rearrange(
                            "p (b mm) -> p b mm", b=2
                        )[:, :, m:m + 1].unsqueeze(3).to_broadcast(
                            (128, 2, 1, D)
                        )
                        pout = onat[:].rearrange(
                            "p (mm b w) -> p mm b w", mm=4, b=4
                        )[:, m:m + 1, 2 * idx:2 * idx + 2, :]
                        nc.vector.tensor_tensor(pout, pin, rzb, ALU.mult)
                    eng = nc.sync if m % 2 == 0 else nc.scalar
                    eng.dma_start_transpose(
                        osb[:, m * 128:(m + 1) * 128],
                        onat[:, m * 128:(m + 1) * 128],
                    )

            with tc.tile_pool(name="po", bufs=2, space="PSUM") as pop:
                po_tiles[0] = (
                    pop.tile([128, 264], F32, name="poA0", tag="po"),
                    pop.tile([128, 264], F32, name="poB0", tag="po"),
                )
                pv_enabled[0] = True
                flush_pv(keep=CPU)
                # emit remaining pass-0 cells (units may straddle into
                # pass 1; their pass-1 PVs wait in pv_pending)
                npass0_cells = NTK * 4
                while cursor[0] < npass0_cells:
                    emit_unit()
                while any(d[0] == 0 for d in pv_pending):
                    emit_pv_cell(pv_pending.pop(0))
                finalize(0)
                po_tiles[1] = (
                    pop.tile([128, 264], F32, name="poA1", tag="po"),
                    pop.tile([128, 264], F32, name="poB1", tag="po"),
                )
                flush_pv(keep=CPU)
                while emit_unit():
                    pass
                flush_pv()
                finalize(1)

                # ---------- tail: Wp projection + bias + out DMA ----------
                pjt = [
                    pop.tile([128, 2 * E], F32, name=f"pjt{i}", tag="po")
                    for i in range(2)
                ]
                # from the st pool: its slots are dead at the tail (the
                # po pool's 2 slots still hold the live pjt tiles)
                bbp = stp.tile([128, E], F32, name="bbp", tag="st")
                for e in range(2):
                    nc.tensor.matmul(
                        bbp[:],
                        bv_col[:, e:e + 1].to_broadcast((128, 128)),
                        w_sb["p"][:, e * E:(e + 1) * E],
                        start=(e == 0),
                        stop=(e == 1),
                    )
                bb_sb = pp.tile([128, E], F32, name="bb_sb")
                nc.vector.tensor_tensor(bb_sb[:], bbp[:], bp_sb[:], ALU.add)
                for m in range(4):
                    for p in range(NPASS):
                        nc.tensor.matmul(
                            pjt[m // 2][:, (m % 2) * E:(m % 2 + 1) * E],
                            osb_t[p][:, m * 128:(m + 1) * 128],
                            w_sb["p"][:, p * E:(p + 1) * E],
                            start=(p == 0 and m % 2 == 0),
                            stop=(p == NPASS - 1 and m % 2 == 1),
                            skip_group_check=True,
                        )
                    if m % 2 == 1:
                        # drain this pjt pair as soon as it stops
                        for mm in (m - 1, m):
                            ob = outp.tile([128, E], F32, tag="outsb")
                            nc.vector.tensor_tensor(
                                ob[:],
                                pjt[mm // 2][:, (mm % 2) * E:(mm % 2 + 1) * E],
                                bb_sb[:], ALU.add,
                            )
                            eng = nc.sync if mm % 2 == 0 else nc.scalar
                            eng.dma_start(
                                out=out[mm * 128:(mm + 1) * 128, :], in_=ob[:]
                            )

    return nc


def get_graph():
    global _GRAPH
    if _GRAPH is None:
        nc = _build_graph()
        nc.compile()
        _GRAPH = nc
    return _GRAPH


def make_in_maps(query, key_, value, Wq, bq, Wk, bk, Wv, bv, Wp, bp):
    query = np.asarray(query, np.float32)
    key_ = np.asarray(key_, np.float32)
    value = np.asarray(value, np.float32)
    Wq, Wk, Wv, Wp = (np.asarray(w, np.float32) for w in (Wq, Wk, Wv, Wp))
    bq, bk, bv, bp = (np.asarray(b_, np.float32) for b_ in (bq, bk, bv, bp))

    wq_b = np.ascontiguousarray(Wq).astype(BF)
    wk_b = np.ascontiguousarray(Wk).astype(BF)
    wv_b = np.ascontiguousarray(Wv).astype(BF)
    wp_b = np.ascontiguousarray(Wp).astype(BF)
    xt = {}
    for b in range(B):
        xt[("q", b)] = np.ascontiguousarray(query[:, b, :].T).astype(BF)
        xt[("k", b)] = np.ascontiguousarray(key_[:, b, :].T).astype(BF)
        xt[("v", b)] = np.ascontiguousarray(value[:, b, :].T).astype(BF)

    in_maps = []
    for c in range(NCORES):
        b = c // SP
        p = c % SP
        m = {
            "xqt": np.ascontiguousarray(xt[("q", b)][:, p * TQ:(p + 1) * TQ]),
            "xkt": xt[("k", b)],
            "xvt": xt[("v", b)],
            "wq": wq_b,
            "wk": wk_b,
            "wv": wv_b,
            "wp": wp_b,
            "bq": bq.reshape(1, E).copy(),
            "bk": bk.reshape(1, E).copy(),
            "bv": bv.reshape(1, E).copy(),
            "bp": bp.reshape(1, E).copy(),
        }
        in_maps.append(m)
    return in_maps


def assemble(results):
    out_full = np.empty((L, B, E), np.float32)
    for c in range(NCORES):
        b = c // SP
        p = c % SP
        out_full[p * TQ:(p + 1) * TQ, b, :] = results[c]["out"]
    return out_full


def run(inputs, trace=False, **kw):
    nc = get_graph()
    in_maps = make_in_maps(**inputs)
    res = run_bass_kernel_spmd(
        nc, in_maps, core_ids=list(range(NCORES)), trace=trace, **kw
    )
    return res


def kernel(**inputs):
    res = run(inputs, trace=False)
    return assemble(res.results)
